# revision 26
# baseline (speedup 1.0000x reference)
"""Trainium2 Bass kernel for Cell2Vec GNN message passing (8 NeuronCores).

Math: 3x GraphConv (DGL norm='both') + node-select + projection + cell-embedding
scores:
    out = emb[c_indices] @ (relu-chain...)  -> [N_C, N_SEL]

Restructure used on device (per layer):
    H_next = relu( Ahat @ (H @ W) + b ),  Ahat = D_in^-1/2 A D_out^-1/2

Sharding: nodes are dst-sharded across 8 cores (6250 each, padded 6272 = 49
tiles of 128). Per layer, each core computes Zhat = scale*(H_own @ W) (dense,
PE), an AllGather replicates Zhat in two row-chunks A/B, then each core
aggregates its owned dst-nodes with one-hot matmuls
    aggT[feat, dstslot] += msg[lane, feat]^T @ Onehot[lane, dstslot]
producing H_next^T (feature-major) = the lhsT layout the next dense needs.

Key tricks (v3):
  - When all biases are zero (they are for this model), relu(nd*G) = nd*relu(G)
    lets every degree-norm scale fold into exact fp32 per-partition scales at
    the Z-cast (scale = ns*nd per src node) and at the final output stage
    (nd per selected column). The one-hot tables then hold EXACT binary 1.0
    values and are stored in fp8 (half the HBM traffic of bf16 w_e tables).
    A general-bias fallback keeps w_e in bf16 tables and unit scales.
  - Padded gather-index slots are -1 (trailing per call): the SWDGE engine
    skips them, cutting ~20% of gather HBM reads and descriptor work. The
    msg buffers are memset once at startup so skipped lanes stay finite
    (they multiply 0.0 one-hot columns).
  - Pipeline: per layer an A-sweep (edge tiles whose src rows are in
    AllGather chunk A) accumulates psum -> fp32->SBUF partial(+bias), then a
    B-sweep finishes the sum, relus into a transient H tile and immediately
    runs the next layer's dense for that bin (skew 2). AllGather chunks fire
    as soon as their Z rows exist.
  - Layer 3 has no AllGather at all: each core aggregates ITS OWN src rows
    (local Z3) into ALL cores' selected-node columns, then one small
    ReduceScatter (fp32, 2 column-halves, overlapped) delivers each owner its
    summed selected columns. The projection + out matmuls are column-major so
    the final nd scale is per-partition.

Bins are in-degree balanced per core (host preprocessing); all 8 cores run
one identical SPMD program. The host reassembles [1024, 8192] from per-core
column blocks.
"""
import heapq
import numpy as np
import ml_dtypes

P = 128
C = 8

# full-problem config (hardcoded per spec; kernel.py must be self-contained)
N_NODES = 50000
N_EDGES = 400000
IN_F = 512
HID = 512
OUT_F = 256
N_CELL = 1000
N_DIM = 128
N_SEL = 8192
N_C = 1024

BF16 = ml_dtypes.bfloat16
FP8 = ml_dtypes.float8_e4m3fn

_COMPILE_CACHE = {}
LAST_EXEC_TIME_NS = None
TRACE = False
PAD_NEG = False     # -1 padded gather indices (SWDGE skips trailing negatives)
GB = 3              # bins per batched gather call (L1/L2 sweeps)
GB3 = 4             # L3 groups per batched gather call


# ----------------------------------------------------------------------------
# host preprocessing
# ----------------------------------------------------------------------------

def _balance_bins(weights, n_bins, cap):
    """Greedy balanced binning: heaviest first into least-loaded open bin.
    Returns (bin_of_item, slot_of_item)."""
    order = np.argsort(-weights, kind="stable")
    heap = [(0.0, b) for b in range(n_bins)]
    heapq.heapify(heap)
    counts = np.zeros(n_bins, np.int64)
    bin_of = np.empty(len(weights), np.int64)
    slot_of = np.empty(len(weights), np.int64)
    for i in order:
        spill = []
        while True:
            load, b = heapq.heappop(heap)
            if counts[b] < cap:
                break
            spill.append((load, b))
        bin_of[i] = b
        slot_of[i] = counts[b]
        counts[b] += 1
        heapq.heappush(heap, (load + float(weights[i]), b))
        for s in spill:
            heapq.heappush(heap, s)
    return bin_of, slot_of


def _wrap_idx(v):
    """[..., L] -> [..., 16, L//16] with unwrapped[j] = w[j%16, j//16]."""
    shp = v.shape[:-1]
    L = v.shape[-1]
    return v.reshape(*shp, L // 16, 16).swapaxes(-1, -2)


def _build_graph_tables(isB, relrow, w_e, qslot, group_c, group_d, C_, ntiles,
                        oh_dt):
    """Per-(core, bin) edge layout for batched dma_gather (A/B split tables).

    Returns Klo, Khi,
      oh   [C, ntiles, P, (Klo+Khi)*P]  (w at (lane, k*P+q)),
      idxw [C, P, ntiles*(Klo+Khi)*8]   int16 wrapped gather indices,
           padded slots = -1 (trailing per (bin,half) -> skipped by SWDGE).
    """
    E = len(relrow)
    hi = np.asarray(isB).astype(np.int64)
    key = (group_c * ntiles + group_d) * 2 + hi
    order = np.argsort(key, kind="stable")
    ks = key[order]
    ngroups = C_ * ntiles * 2
    gs = np.searchsorted(ks, np.arange(ngroups))
    ge = np.searchsorted(ks, np.arange(ngroups), side="right")
    cnt = (ge - gs).reshape(C_, ntiles, 2)
    Klo = max(1, int(np.ceil(cnt[:, :, 0].max() / P)))
    Khi = max(1, int(np.ceil(cnt[:, :, 1].max() / P)))
    K = Klo + Khi

    pos = np.arange(E) - gs[ks]                  # position within (c,d,half)
    cc = ks // (2 * ntiles)
    dd = (ks // 2) % ntiles
    hh = ks % 2
    tile_ = np.where(hh == 0, pos // P, Klo + pos // P)
    lane = pos % P
    es = order

    oh = np.zeros((C_, ntiles, P, K * P), oh_dt)
    oh[cc, dd, lane, tile_ * P + qslot[es]] = w_e[es].astype(oh_dt)

    pad = -1 if PAD_NEG else 0
    ilo = np.full((C_, ntiles, Klo * P), pad, np.int16)
    ihi = np.full((C_, ntiles, Khi * P), pad, np.int16)
    mlo, mhi = hh == 0, hh == 1
    ilo[cc[mlo], dd[mlo], pos[mlo]] = relrow[es[mlo]].astype(np.int16)
    ihi[cc[mhi], dd[mhi], pos[mhi]] = relrow[es[mhi]].astype(np.int16)

    def devidx(v, kk):                            # [C, ntiles, kk*P] -> table
        w = _wrap_idx(v)                          # [C, ntiles, 16, kk*8]
        pc = w.transpose(0, 2, 1, 3).reshape(C_, 16, ntiles * kk * 8)
        return np.ascontiguousarray(np.tile(pc, (1, P // 16, 1)))

    return (Klo, Khi, np.ascontiguousarray(oh),
            devidx(ilo, Klo), devidx(ihi, Khi))


def _build_l3_tables(src_core, grp, localrow_src, w, qslot, NG, C_, oh_dt):
    """Src-sharded layer-3 layout: single local-gather table per (core, grp).
    grp is the emission-ordered global bin id. Padded slots gather row 0
    (weight 0). Returns K3, oh3 [C, NG, P, K3*P], idxw3 [C, P, NG*K3*8]."""
    E = len(grp)
    key = src_core * NG + grp
    order = np.argsort(key, kind="stable")
    ks = key[order]
    gs = np.searchsorted(ks, np.arange(C_ * NG))
    ge = np.searchsorted(ks, np.arange(C_ * NG), side="right")
    cnt = ge - gs
    K3 = max(1, int(np.ceil(cnt.max() / P)))

    pos = np.arange(E) - gs[ks]
    cc = ks // NG
    gg = ks % NG
    tile_ = pos // P
    lane = pos % P
    es = order

    oh = np.zeros((C_, NG, P, K3 * P), oh_dt)
    oh[cc, gg, lane, tile_ * P + qslot[es]] = w[es].astype(oh_dt)
    idx = np.zeros((C_, NG, K3 * P), np.int16)
    idx[cc, gg, pos] = localrow_src[es].astype(np.int16)
    wi = _wrap_idx(idx)                           # [C, NG, 16, K3*8]
    percol = wi.transpose(0, 2, 1, 3).reshape(C_, 16, NG * K3 * 8)
    idxw = np.tile(percol, (1, P // 16, 1))
    return K3, np.ascontiguousarray(oh), np.ascontiguousarray(idxw)


def preprocess(x, src, dst, x_indices, c_indices, nobias):
    src = np.asarray(src).astype(np.int64)
    dst = np.asarray(dst).astype(np.int64)
    x_indices = np.asarray(x_indices).astype(np.int64)
    c_indices = np.asarray(c_indices).astype(np.int64)
    x = np.asarray(x)
    n = x.shape[0]
    nshard = n // C
    nt = (nshard + P - 1) // P
    npad = nt * P
    nt_a = (nt + 1) // 2           # AllGather chunk A = first nt_a tiles
    rows_a, rows_b = nt_a * P, (nt - nt_a) * P
    oh_dt = FP8 if nobias else BF16

    deg_out = np.bincount(src, minlength=n).astype(np.float64)
    deg_in = np.bincount(dst, minlength=n).astype(np.float64)
    ns = np.where(deg_out > 0, 1.0 / np.sqrt(np.maximum(deg_out, 1.0)), 0.0)
    nd = np.where(deg_in > 0, 1.0 / np.sqrt(np.maximum(deg_in, 1.0)), 0.0)
    w_e = (ns[src] * nd[dst]).astype(np.float32)

    owner_n = np.arange(n) // nshard
    localrow = np.empty(n, np.int64)
    for c in range(C):
        nodes = np.arange(c * nshard, (c + 1) * nshard)
        b, s = _balance_bins(deg_in[nodes], nt, P)
        localrow[nodes] = b * P + s
    isB_n = localrow >= rows_a
    relrow_n = np.where(isB_n, owner_n * rows_b + (localrow - rows_a),
                        owner_n * rows_a + localrow)

    # L1/L2 edge layout; binary one-hot when biases are zero (norms folded
    # into exact per-node scales instead)
    w12 = np.ones_like(w_e) if nobias else w_e
    Klo, Khi, oh12, idxwA, idxwB = _build_graph_tables(
        isB_n[src], relrow_n[src], w12, (localrow[dst] % P).astype(np.int64),
        dst // nshard, localrow[dst] // P, C, nt, oh_dt)

    # per-node Z-cast scales, bin layout: [C, 128, nt]
    sc1_n = ns if nobias else np.ones(n)
    sc23_n = (ns * nd) if nobias else np.ones(n)
    sc1 = np.zeros((C, P, nt), np.float32)
    sc23 = np.zeros((C, P, nt), np.float32)
    for c in range(C):
        nodes = np.arange(c * nshard, (c + 1) * nshard)
        lr = localrow[nodes]
        sc1[c, lr % P, lr // P] = sc1_n[nodes]
        sc23[c, lr % P, lr // P] = sc23_n[nodes]

    # L3: src-sharded aggregation into ALL owners' selected columns
    sel_nodes = np.unique(x_indices)
    sel_mask = np.zeros(n, bool)
    sel_mask[sel_nodes] = True
    e3 = np.nonzero(sel_mask[dst])[0]
    deg3 = np.bincount(dst[e3], minlength=n).astype(np.float64)
    ncol_max = max(int((sel_nodes // nshard == c).sum()) for c in range(C))
    T3 = max(1, (ncol_max + P - 1) // P)
    ncol = T3 * P
    colpos = np.full(n, 0, np.int64)
    for c in range(C):
        nodes = sel_nodes[sel_nodes // nshard == c]
        b, s = _balance_bins(deg3[nodes], T3, P)
        colpos[nodes] = b * P + s
    # emission order: lb-major then owner, so column-half 0 completes first
    NG = T3 * C
    lb3 = colpos[dst[e3]] // P
    ow3 = dst[e3] // nshard
    grp3 = lb3 * C + ow3
    w3 = np.ones(len(e3), np.float32) if nobias else w_e[e3]
    K3, oh3, idxw3 = _build_l3_tables(
        src[e3] // nshard, grp3, localrow[src[e3]], w3,
        (colpos[dst[e3]] % P).astype(np.int64), NG, C, oh_dt)

    # per-selected-column nd scale for the output stage: [C, 128, T3]
    nd3 = np.zeros((C, P, T3), np.float32)
    if nobias:
        cp = colpos[sel_nodes]
        nd3[sel_nodes // nshard, cp % P, cp // P] = nd[sel_nodes]
    else:
        nd3[:] = 1.0

    xi_owner = (x_indices // nshard).astype(np.int32)
    xi_col = colpos[x_indices].astype(np.int32)

    # per-core permuted x^T, bin-major: [128, nt, FCH*128]
    F = x.shape[1]
    FC = F // P
    xT = np.zeros((C, P, nt, FC * P), BF16)
    for c in range(C):
        nodes = np.arange(c * nshard, (c + 1) * nshard)
        xv = x[nodes].astype(BF16)            # [nshard, F]
        lr = localrow[nodes]
        bi, sl = lr // P, lr % P
        for fc in range(FC):
            xT[c, :, bi, fc * P + sl] = xv[:, fc * P:(fc + 1) * P]
    xT = xT.reshape(C, P, nt * FC * P)

    return dict(
        n=n, nshard=nshard, nt=nt, npad=npad, T3=T3, ncol=ncol, NG=NG,
        Klo=Klo, Khi=Khi, K3=K3,
        idxwA=idxwA, idxwB=idxwB, idxw3=idxw3, oh12=oh12, oh3=oh3, xT=xT,
        sc1=sc1, sc23=sc23, nd3=nd3,
        xi_owner=xi_owner, xi_col=xi_col,
    )


def _pack_weights(W1, b1, W2, b2, W3, b3, Wp, bp, emb, c_indices):
    """Device layouts: W [fin, fout] -> [128, nchunk*fout]; b -> [128, nchunk];
    bb -> bias broadcast [128, fin_chunks*128] (b[f*128+p] at [p, f*128+j])."""
    def wdev(W):
        fin, fout = W.shape
        nc_ = fin // P
        return np.ascontiguousarray(
            W.astype(BF16).reshape(nc_, P, fout).transpose(1, 0, 2).reshape(P, nc_ * fout))

    def bdev(b):
        nc_ = len(b) // P
        return np.ascontiguousarray(
            np.asarray(b, np.float32).reshape(nc_, P).T)

    def bbcast(b):
        nc_ = len(b) // P
        v = np.asarray(b, np.float32).reshape(nc_, P)     # [f, p]
        return np.ascontiguousarray(
            np.repeat(v.T[:, :, None], P, axis=2).reshape(P, nc_ * P))

    c_idx = np.asarray(c_indices, np.int64)
    ncg = (len(c_idx) + P - 1) // P
    tmp = np.zeros(ncg * P, np.int16)
    tmp[:len(c_idx)] = c_idx
    cidx_dev = np.ascontiguousarray(
        np.tile(tmp.reshape(ncg * 8, 16).T, (P // 16, 1)))
    return dict(
        W1=wdev(W1), W2=wdev(W2), W3=wdev(W3), Wp=wdev(Wp),
        bb1=bbcast(b1), bb2=bbcast(b2), b3=bdev(b3), bp=bdev(bp),
        emb=np.asarray(emb, np.float32), cidx=cidx_dev, ncg=ncg,
    )


# ----------------------------------------------------------------------------
# bass program
# ----------------------------------------------------------------------------

def build_program(meta):
    import concourse.bacc as bacc
    import concourse.bass as bass
    import concourse.mybir as mybir
    import concourse.tile as tile
    from concourse.masks import make_identity

    nt, npad = meta["nt"], meta["npad"]
    T3, ncol, NG = meta["T3"], meta["ncol"], meta["NG"]
    Klo, Khi, K3 = meta["Klo"], meta["Khi"], meta["K3"]
    K = Klo + Khi
    ncg = meta["ncg"]
    hid, out_f = meta["hid"], meta["out_f"]
    n_cell, n_dim, n_c = meta["n_cell"], meta["n_dim"], meta["n_c"]
    nobias = meta["nobias"]
    nt_a = (nt + 1) // 2
    nt_b = nt - nt_a
    rows_a, rows_b = nt_a * P, nt_b * P
    FCH = hid // P            # chunks of hidden width
    FCO = out_f // P          # chunks of layer-3 output width
    lb_h0 = (T3 + 1) // 2     # L3 column-halves (ReduceScatter chunks)
    lb_h1 = T3 - lb_h0
    dt = mybir.dt
    AF = mybir.ActivationFunctionType
    oh_dt = dt.float8e4 if nobias else dt.bfloat16
    SKEW = 2                  # bins between agg-finish and its dense

    nc = bacc.Bacc("TRN2", target_bir_lowering=False, debug=False, num_devices=C,
                   num_swdge_queues=4)

    def din(name, shape, dtype):
        return nc.dram_tensor(name, list(shape), dtype, kind="ExternalInput").ap()

    xT_d = din("xT", (P, nt * FCH * P), dt.bfloat16)
    oh12_d = din("oh12", (nt, P, K * P), oh_dt)
    oh3_d = din("oh3", (NG, P, K3 * P), oh_dt)
    idxwA_d = din("idxwA", (P, nt * Klo * 8), dt.int16)
    idxwB_d = din("idxwB", (P, nt * Khi * 8), dt.int16)
    idxw3_d = din("idxw3", (P, NG * K3 * 8), dt.int16)
    GB = meta["GB"]           # bins per batched gather call
    GB3 = meta["GB3"]         # L3 groups per batched gather call
    cidx_d = din("cidx", (P, ncg * 8), dt.int16)
    W1_d = din("W1", (P, FCH * hid), dt.bfloat16)
    W2_d = din("W2", (P, FCH * hid), dt.bfloat16)
    W3_d = din("W3", (P, FCH * out_f), dt.bfloat16)
    Wp_d = din("Wp", (P, FCO * n_dim), dt.bfloat16)
    bb1_d = din("bb1", (P, FCH * P), dt.float32)
    bb2_d = din("bb2", (P, FCH * P), dt.float32)
    b3_d = din("b3", (P, FCO), dt.float32)
    bp_d = din("bp", (P, 1), dt.float32)
    sc1_d = din("sc1", (P, nt), dt.float32)
    sc23_d = din("sc23", (P, nt), dt.float32)
    nd3_d = din("nd3", (P, T3), dt.float32)
    emb_d = din("emb", (n_cell, n_dim), dt.float32)

    out_d = nc.dram_tensor("outcT", [ncol, n_c], dt.float32,
                           kind="ExternalOutput").ap()

    zfull = [
        (nc.dram_tensor(f"zfullA{i}", [C * rows_a, hid],
                        dt.bfloat16, kind="Internal", addr_space="Shared").ap(),
         nc.dram_tensor(f"zfullB{i}", [C * rows_b, hid],
                        dt.bfloat16, kind="Internal", addr_space="Shared").ap())
        for i in range(2)
    ]
    g3h = [
        nc.dram_tensor("g3h0", [out_f, lb_h0 * P], dt.float32,
                       kind="Internal").ap(),
        nc.dram_tensor("g3h1", [out_f, lb_h1 * P], dt.float32,
                       kind="Internal").ap(),
    ]

    from concourse import library_config

    with tile.TileContext(nc) as tc:
        with tc.tile_pool(name="dram", bufs=1, space="DRAM") as dram, \
             tc.tile_pool(name="persist", bufs=1) as persist, \
             tc.tile_pool(name="xtp", bufs=3) as xtp, \
             tc.tile_pool(name="msgp", bufs=2) as msgp, \
             tc.tile_pool(name="ohp", bufs=4) as ohp, \
             tc.tile_pool(name="htp", bufs=4) as htp, \
             tc.tile_pool(name="tmpp", bufs=2) as tmpp, \
             tc.tile_pool(name="zst", bufs=3) as zst, \
             tc.tile_pool(name="psA", bufs=3, space="PSUM") as psA, \
             tc.tile_pool(name="psB", bufs=2, space="PSUM") as psB, \
             tc.tile_pool(name="psD", bufs=2, space="PSUM") as psD:

            nc.gpsimd.load_library(library_config.mlp)
            gq = [0]          # global SWDGE queue rotation

            def next_q():
                q = gq[0] % 4
                gq[0] += 1
                return q

            # ---- persistent tiles ----
            idxwA_t = persist.tile([P, nt * Klo * 8], dt.int16, tag="gidxA")
            idxwB_t = persist.tile([P, nt * Khi * 8], dt.int16, tag="gidxB")
            idxw3_t = persist.tile([P, NG * K3 * 8], dt.int16, tag="gidx3")
            ident = persist.tile([P, P], dt.float32, tag="ident")
            partial = persist.tile([P, nt, hid], dt.bfloat16, tag="partial")
            H3T = persist.tile([P, FCO, ncol], dt.bfloat16, tag="H3T")
            embT = persist.tile([P, ncg * P], dt.bfloat16, tag="embT")
            projT = persist.tile([P, ncol], dt.bfloat16, tag="projT")
            W1t = persist.tile([P, FCH * hid], dt.bfloat16, tag="W1")
            W2t = persist.tile([P, FCH * hid], dt.bfloat16, tag="W2")
            W3t = persist.tile([P, FCH * out_f], dt.bfloat16, tag="W3")
            Wpt = persist.tile([P, FCO * n_dim], dt.bfloat16, tag="Wp")
            bb1t = persist.tile([P, FCH * P], dt.float32, tag="bb1")
            bb2t = persist.tile([P, FCH * P], dt.float32, tag="bb2")
            b3t = persist.tile([P, FCO], dt.float32, tag="b3")
            bpt = persist.tile([P, 1], dt.float32, tag="bp")
            sc1t = persist.tile([P, nt], dt.float32, tag="sc1")
            sc23t = persist.tile([P, nt], dt.float32, tag="sc23")
            nd3t = persist.tile([P, T3], dt.float32, tag="nd3")
            cidx_t = persist.tile([P, ncg * 8], dt.int16, tag="cidx")
            e_all = persist.tile([P, ncg, n_dim], dt.float32, tag="eg")

            make_identity(nc, ident[:])
            nc.sync.dma_start(idxwA_t[:], idxwA_d[:])
            nc.sync.dma_start(idxwB_t[:], idxwB_d[:])
            nc.sync.dma_start(idxw3_t[:], idxw3_d[:])
            nc.sync.dma_start(cidx_t[:], cidx_d[:])
            nc.sync.dma_start(W1t[:], W1_d[:])
            nc.sync.dma_start(sc1t[:], sc1_d[:])
            nc.sync.dma_start(sc23t[:], sc23_d[:])
            nc.sync.dma_start(W2t[:], W2_d[:])
            nc.sync.dma_start(bb1t[:], bb1_d[:])
            nc.sync.dma_start(bb2t[:], bb2_d[:])
            nc.sync.dma_start(W3t[:], W3_d[:])
            nc.sync.dma_start(b3t[:], b3_d[:])
            nc.sync.dma_start(Wpt[:], Wp_d[:])
            nc.sync.dma_start(bpt[:], bp_d[:])
            nc.sync.dma_start(nd3t[:], nd3_d[:])

            # ---- cell-embedding gather + transpose (under AllGather-1) ----
            nc.gpsimd.dma_gather(
                e_all[:], emb_d[:], cidx_t[:], ncg * P, ncg * P, n_dim,
                queue_num=next_q())
            for g in range(ncg):
                pt = psD.tile([P, 512], dt.float32, space="PSUM", tag="pd")
                nc.tensor.transpose(pt[:, 0:P], e_all[:, g, :], ident[:])
                nc.vector.tensor_copy(embT[:, g * P:(g + 1) * P], pt[:, 0:P])

            # per-layer AllGather chunk tiles + local Z3
            zchunks = []
            for l in range(2):
                zca = dram.tile([rows_a, hid], dt.bfloat16, tag=f"zca{l}")
                zcb = dram.tile([rows_b, hid], dt.bfloat16, tag=f"zcb{l}")
                zchunks.append((zca, zcb))
            z3loc = dram.tile([npad, out_f], dt.bfloat16, tag="z3loc")
            ph0 = dram.tile([C, out_f, lb_h0 * P], dt.float32, tag="ph0")
            ph1 = dram.tile([C, out_f, lb_h1 * P], dt.float32, tag="ph1")

            def fire_ag(l, half):
                zc = zchunks[l][half]
                nc.gpsimd.collective_compute(
                    "AllGather", mybir.AluOpType.bypass,
                    replica_groups=[list(range(C))],
                    ins=[zc[:]], outs=[zfull[l][half]])

            def dense_bin(l, i, lhsT_tile):
                """Zhat_l tile for bin i: scale_l * (H_bin @ W_l)."""
                fout = hid if l < 2 else out_f
                Wt = (W1t, W2t, W3t)[l]
                sct = sc1t if l == 0 else sc23t
                ps = psD.tile([P, 512], dt.float32, space="PSUM", tag="pd")
                for f in range(FCH):
                    nc.tensor.matmul(
                        ps[:, :fout],
                        lhsT=lhsT_tile[:, f * P:(f + 1) * P],
                        rhs=Wt[:, f * fout:(f + 1) * fout],
                        start=(f == 0), stop=(f == FCH - 1))
                zs = zst.tile([P, 512], dt.bfloat16, tag="zs")
                nc.scalar.activation(zs[:, :fout], ps[:, :fout], AF.Identity,
                                     scale=sct[:, i:i + 1])
                if l == 2:
                    nc.sync.dma_start(z3loc[i * P:(i + 1) * P, :], zs[:, :fout])
                    return
                if i < nt_a:
                    nc.sync.dma_start(zchunks[l][0][i * P:(i + 1) * P, :],
                                      zs[:, :fout])
                else:
                    j = i - nt_a
                    nc.sync.dma_start(zchunks[l][1][j * P:(j + 1) * P, :],
                                      zs[:, :fout])
                if i == nt_a - 1:
                    fire_ag(l, 0)
                elif i == nt - 1:
                    fire_ag(l, 1)

            def gather_group(zf, idx_t, kk, d0, g, tag, msg_shape):
                """One batched dma_gather covering bins [d0, d0+g)."""
                msg = msgp.tile(msg_shape, dt.bfloat16, tag=tag)
                nc.gpsimd.dma_gather(
                    msg[:, 0:g * kk, :], zf[:],
                    idx_t[:, d0 * kk * 8: (d0 + g) * kk * 8],
                    g * kk * P, g * kk * P, msg_shape[2], queue_num=next_q())
                return msg

            def sweep_A(l, bb_t):
                """A-half matmuls for every bin -> bf16 partial (+bias)."""
                zfa = zfull[l][0]
                for d0 in range(0, nt, GB):
                    g = min(GB, nt - d0)
                    tag = "msgA" if g == GB else "msgAr"
                    msg = gather_group(zfa, idxwA_t, Klo, d0, g, tag,
                                       [P, g * Klo, FCH * P])
                    for j in range(g):
                        d = d0 + j
                        oh_t = ohp.tile([P, Klo * P], oh_dt, tag="ohA")
                        nc.sync.dma_start(oh_t[:], oh12_d[d][:, 0:Klo * P])
                        ps = psA.tile([P, 512], dt.float32, space="PSUM",
                                      tag="pa")
                        for f in range(FCH):
                            for k in range(Klo):
                                nc.tensor.matmul(
                                    ps[:, f * P:(f + 1) * P],
                                    lhsT=msg[:, j * Klo + k, f * P:(f + 1) * P],
                                    rhs=oh_t[:, k * P:(k + 1) * P],
                                    start=(k == 0), stop=(k == Klo - 1))
                        nc.vector.tensor_add(partial[:, d, :], ps[:], bb_t[:])

            def sweep_B(l, consume):
                """B-half matmuls + partial add + relu -> transient H tile;
                calls consume(i, ht_tile) skewed by SKEW bins."""
                zfb = zfull[l][1]
                pending = []
                for d0 in range(0, nt, GB):
                    g = min(GB, nt - d0)
                    tag = "msgB" if g == GB else "msgBr"
                    msg = gather_group(zfb, idxwB_t, Khi, d0, g, tag,
                                       [P, g * Khi, FCH * P])
                    for j in range(g):
                        d = d0 + j
                        oh_t = ohp.tile([P, Khi * P], oh_dt, tag="ohB")
                        nc.sync.dma_start(oh_t[:], oh12_d[d][:, Klo * P:K * P])
                        ps = psB.tile([P, 512], dt.float32, space="PSUM",
                                      tag="pb")
                        for f in range(FCH):
                            for k in range(Khi):
                                nc.tensor.matmul(
                                    ps[:, f * P:(f + 1) * P],
                                    lhsT=msg[:, j * Khi + k, f * P:(f + 1) * P],
                                    rhs=oh_t[:, k * P:(k + 1) * P],
                                    start=(k == 0), stop=(k == Khi - 1))
                        tmp = tmpp.tile([P, 512], dt.float32, tag="tmp")
                        nc.vector.tensor_add(tmp[:], ps[:], partial[:, d, :])
                        ht = htp.tile([P, FCH * P], dt.bfloat16, tag="ht")
                        nc.scalar.activation(ht[:], tmp[:], AF.Relu)
                        pending.append((d, ht))
                        if len(pending) > SKEW:
                            consume(*pending.pop(0))
                for item in pending:
                    consume(*item)

            # ---- L1 dense sweep (from streamed xT tiles) ----
            for i in range(nt):
                xt = xtp.tile([P, FCH * P], dt.bfloat16, tag="xt")
                nc.sync.dma_start(xt[:], xT_d[:, i * FCH * P:(i + 1) * FCH * P])
                dense_bin(0, i, xt)

            # ---- layers 1 and 2: aggregate + next dense interleaved ----
            for l in range(2):
                sweep_A(l, (bb1t, bb2t)[l])
                sweep_B(l, lambda i, ht, _l=l: dense_bin(_l + 1, i, ht))

            # ---- layer 3: src-sharded local aggregation + ReduceScatter ----
            # groups emitted lb-major: grp = lb*C + owner
            for g0 in range(0, NG, GB3):
                msg = msgp.tile([P, GB3 * K3, out_f], dt.bfloat16, tag="msg3")
                nc.gpsimd.dma_gather(
                    msg[:], z3loc[:],
                    idxw3_t[:, g0 * K3 * 8: (g0 + GB3) * K3 * 8],
                    GB3 * K3 * P, GB3 * K3 * P, out_f, queue_num=next_q())
                for j in range(GB3):
                    g = g0 + j
                    lb, ow = g // C, g % C
                    oh_t = ohp.tile([P, K3 * P], oh_dt, tag="oh3")
                    nc.sync.dma_start(oh_t[:], oh3_d[g])
                    ps = psA.tile([P, 512], dt.float32, space="PSUM", tag="pa")
                    for f in range(FCO):
                        for k in range(K3):
                            nc.tensor.matmul(
                                ps[:, f * P:(f + 1) * P],
                                lhsT=msg[:, j * K3 + k, f * P:(f + 1) * P],
                                rhs=oh_t[:, k * P:(k + 1) * P],
                                start=(k == 0), stop=(k == K3 - 1))
                    p3 = zst.tile([P, 512], dt.float32, tag="os")
                    nc.vector.tensor_copy(p3[:, :out_f], ps[:, :out_f])
                    ph, lbr = (ph0, lb) if lb < lb_h0 else (ph1, lb - lb_h0)
                    for f in range(FCO):
                        nc.sync.dma_start(
                            ph[ow, f * P:(f + 1) * P, lbr * P:(lbr + 1) * P],
                            p3[:, f * P:(f + 1) * P])
                    if g == lb_h0 * C - 1:
                        nc.gpsimd.collective_compute(
                            "ReduceScatter", mybir.AluOpType.add,
                            replica_groups=[list(range(C))],
                            ins=[ph0[:]], outs=[g3h[0]])
                    elif g == NG - 1:
                        nc.gpsimd.collective_compute(
                            "ReduceScatter", mybir.AluOpType.add,
                            replica_groups=[list(range(C))],
                            ins=[ph1[:]], outs=[g3h[1]])

            # ---- post-RS: relu(+b3) -> H3T; projection per column-half ----
            nseg = (ncol + 511) // 512
            col_base = [0, lb_h0 * P]
            for h in range(2):
                wcols = (lb_h0 if h == 0 else lb_h1) * P
                for f in range(FCO):
                    g3sb = tmpp.tile([P, 512], dt.float32, tag="g3")
                    nc.sync.dma_start(g3sb[:, :wcols],
                                      g3h[h][f * P:(f + 1) * P, :])
                    nc.scalar.activation(
                        H3T[:, f, col_base[h]:col_base[h] + wcols],
                        g3sb[:, :wcols], AF.Relu, bias=b3t[:, f:f + 1])
            for s in range(nseg):
                w = min(512, ncol - s * 512)
                pp = psD.tile([P, 512], dt.float32, space="PSUM", tag="pd")
                for f in range(FCO):
                    nc.tensor.matmul(
                        pp[:, :w],
                        lhsT=Wpt[:, f * n_dim:(f + 1) * n_dim],
                        rhs=H3T[:, f, s * 512: s * 512 + w],
                        start=(f == 0), stop=(f == FCO - 1))
                nc.scalar.activation(projT[:, s * 512:s * 512 + w], pp[:, :w],
                                     AF.Identity, bias=bpt[:, 0:1])

            # ---- outT[col, cell] = nd3[col] * projT^T @ embT ----
            ncseg = (ncg * P + 511) // 512
            for cb in range(ncol // P):
                for s in range(ncseg):
                    w = min(512, ncg * P - s * 512)
                    po = psD.tile([P, 512], dt.float32, space="PSUM", tag="pd")
                    nc.tensor.matmul(
                        po[:, :w],
                        lhsT=projT[:, cb * P:(cb + 1) * P],
                        rhs=embT[:, s * 512:s * 512 + w],
                        start=True, stop=True)
                    os_ = zst.tile([P, 512], dt.float32, tag="os")
                    nc.vector.tensor_scalar_mul(os_[:, :w], po[:, :w],
                                                nd3t[:, cb:cb + 1])
                    nc.sync.dma_start(
                        out_d[cb * P:(cb + 1) * P, s * 512:s * 512 + w],
                        os_[:, :w])

    nc.compile()
    return nc


# ----------------------------------------------------------------------------
# entry point
# ----------------------------------------------------------------------------

def _ensure_ntff_hook():
    """Register the axon NTFF-profile hook if the image's antenv lacks it.
    Only used on the TRACE path (benchmarking); grading runs trace=False."""
    import sys
    import types
    try:
        from antenv.axon_hooks import get_axon_ntff_profile_hook  # noqa: F401
        return
    except ImportError:
        pass
    try:
        from trn_agent_boot.trn_boot import _ntff_profile_via_ctypes
        hook = _ntff_profile_via_ctypes("/opt/axon/libaxon_pjrt.so")
    except Exception:
        hook = None
    mod = types.ModuleType("antenv.axon_hooks")
    mod._hook = hook
    mod.get_axon_ntff_profile_hook = lambda: mod._hook
    mod.set_axon_ntff_profile_hook = lambda h: setattr(mod, "_hook", h)
    import antenv
    antenv.axon_hooks = mod
    sys.modules["antenv.axon_hooks"] = mod


def kernel(**inputs):
    global LAST_EXEC_TIME_NS
    from concourse import bass_utils
    if TRACE:
        _ensure_ntff_hook()

    x = np.asarray(inputs["x"], np.float32)
    b1 = np.asarray(inputs["b1"], np.float32)
    b2 = np.asarray(inputs["b2"], np.float32)
    b3 = np.asarray(inputs["b3"], np.float32)
    bp = np.asarray(inputs["bp"], np.float32)
    nobias = not (np.any(b1) or np.any(b2) or np.any(b3) or np.any(bp))

    prep = preprocess(x, inputs["src"], inputs["dst"],
                      inputs["x_indices"], inputs["c_indices"], nobias)
    wp = _pack_weights(inputs["W1"], b1, inputs["W2"], b2,
                       inputs["W3"], b3, inputs["Wp"], bp,
                       inputs["emb"], inputs["c_indices"])

    hid = np.asarray(inputs["W1"]).shape[1]
    out_f = np.asarray(inputs["W3"]).shape[1]
    n_dim = np.asarray(inputs["Wp"]).shape[1]
    n_cell = np.asarray(inputs["emb"]).shape[0]
    n_c = len(np.asarray(inputs["c_indices"]))
    meta = dict(nt=prep["nt"], npad=prep["npad"],
                Klo=prep["Klo"], Khi=prep["Khi"], K3=prep["K3"],
                T3=prep["T3"], ncol=prep["ncol"], NG=prep["NG"],
                ncg=wp["ncg"], nobias=nobias, GB=GB, GB3=GB3,
                hid=hid, out_f=out_f, n_dim=n_dim, n_cell=n_cell, n_c=n_c)
    meta_key = tuple(sorted(meta.items()))
    if meta_key not in _COMPILE_CACHE:
        _COMPILE_CACHE[meta_key] = build_program(meta)
    nc = _COMPILE_CACHE[meta_key]

    in_maps = []
    for c in range(C):
        in_maps.append({
            "xT": prep["xT"][c],
            "oh12": prep["oh12"][c],
            "oh3": prep["oh3"][c],
            "idxwA": prep["idxwA"][c],
            "idxwB": prep["idxwB"][c],
            "idxw3": prep["idxw3"][c],
            "sc1": prep["sc1"][c], "sc23": prep["sc23"][c],
            "nd3": prep["nd3"][c],
            "cidx": wp["cidx"],
            "W1": wp["W1"], "W2": wp["W2"], "W3": wp["W3"], "Wp": wp["Wp"],
            "bb1": wp["bb1"], "bb2": wp["bb2"], "b3": wp["b3"], "bp": wp["bp"],
            "emb": wp["emb"],
        })

    # transient NRT_EXEC_UNIT_UNRECOVERABLE flakes recover on a fresh attempt
    last_err = None
    for _attempt in range(3):
        try:
            res = bass_utils.run_bass_kernel_spmd(
                nc, in_maps, core_ids=list(range(C)), trace=TRACE)
            break
        except Exception as e:
            last_err = e
    else:
        raise last_err
    LAST_EXEC_TIME_NS = res.exec_time_ns
    globals()["LAST_RESULTS"] = res

    outs = np.stack([r["outcT"] for r in res.results])    # [C, ncol, N_C]
    final = outs[prep["xi_owner"], prep["xi_col"], :]     # [N_SEL, N_C]
    return np.ascontiguousarray(final.T, np.float32)      # [N_C, N_SEL]


# revision 35
# speedup vs baseline: 1.1920x; 1.1920x over previous
"""Trainium2 Bass kernel for Cell2Vec GNN message passing (8 NeuronCores).

Math: 3x GraphConv (DGL norm='both') + node-select + projection + cell-embedding
scores:
    out = emb[c_indices] @ (relu-chain...)  -> [N_C, N_SEL]

Restructure used on device (per layer):
    H_next = relu( Ahat @ (H @ W) + b ),  Ahat = D_in^-1/2 A D_out^-1/2

Sharding: nodes are dst-sharded across 8 cores (6250 each, padded 6272 = 49
tiles of 128). Per layer, each core computes Zhat = scale*(H_own @ W) (dense,
PE), an AllGather replicates Zhat in two row-chunks A/B, then each core
aggregates its owned dst-nodes with one-hot matmuls
    aggT[feat, dstslot] += msg[lane, feat]^T @ Onehot[lane, dstslot]
producing H_next^T (feature-major) = the lhsT layout the next dense needs.

Key tricks (v3):
  - When all biases are zero (they are for this model), relu(nd*G) = nd*relu(G)
    lets every degree-norm scale fold into exact fp32 per-partition scales at
    the Z-cast (scale = ns*nd per src node) and at the final output stage
    (nd per selected column). The one-hot tables then hold EXACT binary 1.0
    values and are stored in fp8 (half the HBM traffic of bf16 w_e tables).
    A general-bias fallback keeps w_e in bf16 tables and unit scales.
  - Padded gather-index slots are -1 (trailing per call): the SWDGE engine
    skips them, cutting ~20% of gather HBM reads and descriptor work. The
    msg buffers are memset once at startup so skipped lanes stay finite
    (they multiply 0.0 one-hot columns).
  - Pipeline: per layer an A-sweep (edge tiles whose src rows are in
    AllGather chunk A) accumulates psum -> fp32->SBUF partial(+bias), then a
    B-sweep finishes the sum, relus into a transient H tile and immediately
    runs the next layer's dense for that bin (skew 2). AllGather chunks fire
    as soon as their Z rows exist.
  - Layer 3 has no AllGather at all: each core aggregates ITS OWN src rows
    (local Z3) into ALL cores' selected-node columns, then one small
    ReduceScatter (fp32, 2 column-halves, overlapped) delivers each owner its
    summed selected columns. The projection + out matmuls are column-major so
    the final nd scale is per-partition.

Bins are in-degree balanced per core (host preprocessing); all 8 cores run
one identical SPMD program. The host reassembles [1024, 8192] from per-core
column blocks.
"""
import heapq
import numpy as np
import ml_dtypes

P = 128
C = 8

# full-problem config (hardcoded per spec; kernel.py must be self-contained)
N_NODES = 50000
N_EDGES = 400000
IN_F = 512
HID = 512
OUT_F = 256
N_CELL = 1000
N_DIM = 128
N_SEL = 8192
N_C = 1024

BF16 = ml_dtypes.bfloat16
FP8 = ml_dtypes.float8_e4m3fn

_COMPILE_CACHE = {}
LAST_EXEC_TIME_NS = None
TRACE = False
PAD_NEG = False     # -1 padded gather indices (SWDGE skips trailing negatives)
GB = 3              # bins per batched gather call (L1/L2 sweeps)
GB3 = 4             # L3 groups per batched gather call


# ----------------------------------------------------------------------------
# host preprocessing
# ----------------------------------------------------------------------------

def _balance_bins(weights, n_bins, cap):
    """Greedy balanced binning: heaviest first into least-loaded open bin.
    Returns (bin_of_item, slot_of_item)."""
    order = np.argsort(-weights, kind="stable")
    heap = [(0.0, b) for b in range(n_bins)]
    heapq.heapify(heap)
    counts = np.zeros(n_bins, np.int64)
    bin_of = np.empty(len(weights), np.int64)
    slot_of = np.empty(len(weights), np.int64)
    for i in order:
        spill = []
        while True:
            load, b = heapq.heappop(heap)
            if counts[b] < cap:
                break
            spill.append((load, b))
        bin_of[i] = b
        slot_of[i] = counts[b]
        counts[b] += 1
        heapq.heappush(heap, (load + float(weights[i]), b))
        for s in spill:
            heapq.heappush(heap, s)
    return bin_of, slot_of


def _wrap_idx(v):
    """[..., L] -> [..., 16, L//16] with unwrapped[j] = w[j%16, j//16]."""
    shp = v.shape[:-1]
    L = v.shape[-1]
    return v.reshape(*shp, L // 16, 16).swapaxes(-1, -2)


def _build_graph_tables(isB, relrow, w_e, qslot, group_c, group_d, C_, ntiles,
                        oh_dt):
    """Per-(core, bin) edge layout for batched dma_gather (A/B split tables).

    Returns Klo, Khi,
      oh   [C, ntiles, P, (Klo+Khi)*P]  (w at (lane, k*P+q)),
      idxw [C, P, ntiles*(Klo+Khi)*8]   int16 wrapped gather indices,
           padded slots = -1 (trailing per (bin,half) -> skipped by SWDGE).
    """
    E = len(relrow)
    hi = np.asarray(isB).astype(np.int64)
    key = (group_c * ntiles + group_d) * 2 + hi
    order = np.argsort(key, kind="stable")
    ks = key[order]
    ngroups = C_ * ntiles * 2
    gs = np.searchsorted(ks, np.arange(ngroups))
    ge = np.searchsorted(ks, np.arange(ngroups), side="right")
    cnt = (ge - gs).reshape(C_, ntiles, 2)
    Klo = max(1, int(np.ceil(cnt[:, :, 0].max() / P)))
    Khi = max(1, int(np.ceil(cnt[:, :, 1].max() / P)))
    K = Klo + Khi

    pos = np.arange(E) - gs[ks]                  # position within (c,d,half)
    cc = ks // (2 * ntiles)
    dd = (ks // 2) % ntiles
    hh = ks % 2
    tile_ = np.where(hh == 0, pos // P, Klo + pos // P)
    lane = pos % P
    es = order

    oh = np.zeros((C_, ntiles, P, K * P), oh_dt)
    oh[cc, dd, lane, tile_ * P + qslot[es]] = w_e[es].astype(oh_dt)

    pad = -1 if PAD_NEG else 0
    ilo = np.full((C_, ntiles, Klo * P), pad, np.int16)
    ihi = np.full((C_, ntiles, Khi * P), pad, np.int16)
    mlo, mhi = hh == 0, hh == 1
    ilo[cc[mlo], dd[mlo], pos[mlo]] = relrow[es[mlo]].astype(np.int16)
    ihi[cc[mhi], dd[mhi], pos[mhi]] = relrow[es[mhi]].astype(np.int16)

    def devidx(v, kk):                            # [C, ntiles, kk*P] -> table
        w = _wrap_idx(v)                          # [C, ntiles, 16, kk*8]
        pc = w.transpose(0, 2, 1, 3).reshape(C_, 16, ntiles * kk * 8)
        return np.ascontiguousarray(np.tile(pc, (1, P // 16, 1)))

    return (Klo, Khi, np.ascontiguousarray(oh),
            devidx(ilo, Klo), devidx(ihi, Khi))


def _build_l3_tables(src_core, grp, localrow_src, w, qslot, NG, C_, oh_dt):
    """Src-sharded layer-3 layout: single local-gather table per (core, grp).
    grp is the emission-ordered global bin id. Padded slots gather row 0
    (weight 0). Returns K3, oh3 [C, NG, P, K3*P], idxw3 [C, P, NG*K3*8]."""
    E = len(grp)
    key = src_core * NG + grp
    order = np.argsort(key, kind="stable")
    ks = key[order]
    gs = np.searchsorted(ks, np.arange(C_ * NG))
    ge = np.searchsorted(ks, np.arange(C_ * NG), side="right")
    cnt = ge - gs
    K3 = max(1, int(np.ceil(cnt.max() / P)))

    pos = np.arange(E) - gs[ks]
    cc = ks // NG
    gg = ks % NG
    tile_ = pos // P
    lane = pos % P
    es = order

    oh = np.zeros((C_, NG, P, K3 * P), oh_dt)
    oh[cc, gg, lane, tile_ * P + qslot[es]] = w[es].astype(oh_dt)
    idx = np.zeros((C_, NG, K3 * P), np.int16)
    idx[cc, gg, pos] = localrow_src[es].astype(np.int16)
    wi = _wrap_idx(idx)                           # [C, NG, 16, K3*8]
    percol = wi.transpose(0, 2, 1, 3).reshape(C_, 16, NG * K3 * 8)
    idxw = np.tile(percol, (1, P // 16, 1))
    return K3, np.ascontiguousarray(oh), np.ascontiguousarray(idxw)


def preprocess(x, src, dst, x_indices, c_indices, nobias):
    src = np.asarray(src).astype(np.int64)
    dst = np.asarray(dst).astype(np.int64)
    x_indices = np.asarray(x_indices).astype(np.int64)
    c_indices = np.asarray(c_indices).astype(np.int64)
    x = np.asarray(x)
    n = x.shape[0]
    nshard = n // C
    nt = (nshard + P - 1) // P
    npad = nt * P
    nt_a = (nt + 1) // 2           # AllGather chunk A = first nt_a tiles
    rows_a, rows_b = nt_a * P, (nt - nt_a) * P
    oh_dt = FP8 if nobias else BF16

    deg_out = np.bincount(src, minlength=n).astype(np.float64)
    deg_in = np.bincount(dst, minlength=n).astype(np.float64)
    ns = np.where(deg_out > 0, 1.0 / np.sqrt(np.maximum(deg_out, 1.0)), 0.0)
    nd = np.where(deg_in > 0, 1.0 / np.sqrt(np.maximum(deg_in, 1.0)), 0.0)
    w_e = (ns[src] * nd[dst]).astype(np.float32)

    owner_n = np.arange(n) // nshard
    localrow = np.empty(n, np.int64)
    for c in range(C):
        nodes = np.arange(c * nshard, (c + 1) * nshard)
        b, s = _balance_bins(deg_in[nodes], nt, P)
        localrow[nodes] = b * P + s
    isB_n = localrow >= rows_a
    relrow_n = np.where(isB_n, owner_n * rows_b + (localrow - rows_a),
                        owner_n * rows_a + localrow)

    # L1/L2 edge layout; binary one-hot when biases are zero (norms folded
    # into exact per-node scales instead)
    w12 = np.ones_like(w_e) if nobias else w_e
    Klo, Khi, oh12, idxwA, idxwB = _build_graph_tables(
        isB_n[src], relrow_n[src], w12, (localrow[dst] % P).astype(np.int64),
        dst // nshard, localrow[dst] // P, C, nt, oh_dt)

    # per-node Z-cast scales, bin layout: [C, 128, nt]
    sc1_n = ns if nobias else np.ones(n)
    sc23_n = (ns * nd) if nobias else np.ones(n)
    sc1 = np.zeros((C, P, nt), np.float32)
    sc23 = np.zeros((C, P, nt), np.float32)
    for c in range(C):
        nodes = np.arange(c * nshard, (c + 1) * nshard)
        lr = localrow[nodes]
        sc1[c, lr % P, lr // P] = sc1_n[nodes]
        sc23[c, lr % P, lr // P] = sc23_n[nodes]

    # L3: src-sharded aggregation into ALL owners' selected columns
    sel_nodes = np.unique(x_indices)
    sel_mask = np.zeros(n, bool)
    sel_mask[sel_nodes] = True
    e3 = np.nonzero(sel_mask[dst])[0]
    deg3 = np.bincount(dst[e3], minlength=n).astype(np.float64)
    ncol_max = max(int((sel_nodes // nshard == c).sum()) for c in range(C))
    T3 = max(1, (ncol_max + P - 1) // P)
    ncol = T3 * P
    colpos = np.full(n, 0, np.int64)
    for c in range(C):
        nodes = sel_nodes[sel_nodes // nshard == c]
        b, s = _balance_bins(deg3[nodes], T3, P)
        colpos[nodes] = b * P + s
    # emission order: lb-major then owner, so column-half 0 completes first
    NG = T3 * C
    lb3 = colpos[dst[e3]] // P
    ow3 = dst[e3] // nshard
    grp3 = lb3 * C + ow3
    w3 = np.ones(len(e3), np.float32) if nobias else w_e[e3]
    K3, oh3, idxw3 = _build_l3_tables(
        src[e3] // nshard, grp3, localrow[src[e3]], w3,
        (colpos[dst[e3]] % P).astype(np.int64), NG, C, oh_dt)

    # per-selected-column nd scale for the output stage: [C, 128, T3]
    nd3 = np.zeros((C, P, T3), np.float32)
    if nobias:
        cp = colpos[sel_nodes]
        nd3[sel_nodes // nshard, cp % P, cp // P] = nd[sel_nodes]
    else:
        nd3[:] = 1.0

    xi_owner = (x_indices // nshard).astype(np.int32)
    xi_col = colpos[x_indices].astype(np.int32)

    # per-core permuted x^T, bin-major: [128, nt, FCH*128]
    F = x.shape[1]
    FC = F // P
    xT = np.zeros((C, P, nt, FC * P), BF16)
    for c in range(C):
        nodes = np.arange(c * nshard, (c + 1) * nshard)
        xv = x[nodes].astype(BF16)            # [nshard, F]
        lr = localrow[nodes]
        bi, sl = lr // P, lr % P
        for fc in range(FC):
            xT[c, :, bi, fc * P + sl] = xv[:, fc * P:(fc + 1) * P]
    xT = xT.reshape(C, P, nt * FC * P)

    return dict(
        n=n, nshard=nshard, nt=nt, npad=npad, T3=T3, ncol=ncol, NG=NG,
        Klo=Klo, Khi=Khi, K3=K3,
        idxwA=idxwA, idxwB=idxwB, idxw3=idxw3, oh12=oh12, oh3=oh3, xT=xT,
        sc1=sc1, sc23=sc23, nd3=nd3,
        xi_owner=xi_owner, xi_col=xi_col,
    )


def _pack_weights(W1, b1, W2, b2, W3, b3, Wp, bp, emb, c_indices):
    """Device layouts: W [fin, fout] -> [128, nchunk*fout]; b -> [128, nchunk];
    bb -> bias broadcast [128, fin_chunks*128] (b[f*128+p] at [p, f*128+j])."""
    def wdev(W):
        fin, fout = W.shape
        nc_ = fin // P
        return np.ascontiguousarray(
            W.astype(BF16).reshape(nc_, P, fout).transpose(1, 0, 2).reshape(P, nc_ * fout))

    def bdev(b):
        nc_ = len(b) // P
        return np.ascontiguousarray(
            np.asarray(b, np.float32).reshape(nc_, P).T)

    c_idx = np.asarray(c_indices, np.int64)
    ncg = (len(c_idx) + P - 1) // P
    tmp = np.zeros(ncg * P, np.int16)
    tmp[:len(c_idx)] = c_idx
    cidx_dev = np.ascontiguousarray(
        np.tile(tmp.reshape(ncg * 8, 16).T, (P // 16, 1)))
    return dict(
        W1=wdev(W1), W2=wdev(W2), W3=wdev(W3), Wp=wdev(Wp),
        b1=bdev(b1), b2=bdev(b2), b3=bdev(b3), bp=bdev(bp),
        emb=np.asarray(emb, np.float32), cidx=cidx_dev, ncg=ncg,
    )


# ----------------------------------------------------------------------------
# bass program
# ----------------------------------------------------------------------------

def build_program(meta):
    import concourse.bacc as bacc
    import concourse.bass as bass
    import concourse.mybir as mybir
    import concourse.tile as tile
    from concourse.masks import make_identity

    nt, npad = meta["nt"], meta["npad"]
    T3, ncol, NG = meta["T3"], meta["ncol"], meta["NG"]
    Klo, Khi, K3 = meta["Klo"], meta["Khi"], meta["K3"]
    K = Klo + Khi
    ncg = meta["ncg"]
    hid, out_f = meta["hid"], meta["out_f"]
    n_cell, n_dim, n_c = meta["n_cell"], meta["n_dim"], meta["n_c"]
    nobias = meta["nobias"]
    nt_a = (nt + 1) // 2
    nt_b = nt - nt_a
    rows_a, rows_b = nt_a * P, nt_b * P
    FCH = hid // P            # chunks of hidden width
    FCO = out_f // P          # chunks of layer-3 output width
    lb_h0 = (T3 + 1) // 2     # L3 column-halves (ReduceScatter chunks)
    lb_h1 = T3 - lb_h0
    dt = mybir.dt
    AF = mybir.ActivationFunctionType
    oh_dt = dt.float8e4 if nobias else dt.bfloat16
    SKEW = 2                  # bins between agg-finish and its dense

    nc = bacc.Bacc("TRN2", target_bir_lowering=False, debug=False, num_devices=C,
                   num_swdge_queues=4)

    def din(name, shape, dtype):
        return nc.dram_tensor(name, list(shape), dtype, kind="ExternalInput").ap()

    xT_d = din("xT", (P, nt * FCH * P), dt.bfloat16)
    oh12_d = din("oh12", (nt, P, K * P), oh_dt)
    oh3_d = din("oh3", (NG, P, K3 * P), oh_dt)
    idxwA_d = din("idxwA", (P, nt * Klo * 8), dt.int16)
    idxwB_d = din("idxwB", (P, nt * Khi * 8), dt.int16)
    idxw3_d = din("idxw3", (P, NG * K3 * 8), dt.int16)
    GB = meta["GB"]           # bins per batched gather call
    GB3 = meta["GB3"]         # L3 groups per batched gather call
    cidx_d = din("cidx", (P, ncg * 8), dt.int16)
    W1_d = din("W1", (P, FCH * hid), dt.bfloat16)
    W2_d = din("W2", (P, FCH * hid), dt.bfloat16)
    W3_d = din("W3", (P, FCH * out_f), dt.bfloat16)
    Wp_d = din("Wp", (P, FCO * n_dim), dt.bfloat16)
    b1_d = din("b1", (P, FCH), dt.float32)
    b2_d = din("b2", (P, FCH), dt.float32)
    b3_d = din("b3", (P, FCO), dt.float32)
    bp_d = din("bp", (P, 1), dt.float32)
    sc1_d = din("sc1", (P, nt), dt.float32)
    sc23_d = din("sc23", (P, nt), dt.float32)
    nd3_d = din("nd3", (P, T3), dt.float32)
    emb_d = din("emb", (n_cell, n_dim), dt.float32)

    out_d = nc.dram_tensor("outcT", [ncol, n_c], dt.float32,
                           kind="ExternalOutput").ap()

    zfull = [
        (nc.dram_tensor(f"zfullA{i}", [C * rows_a, hid],
                        dt.bfloat16, kind="Internal", addr_space="Shared").ap(),
         nc.dram_tensor(f"zfullB{i}", [C * rows_b, hid],
                        dt.bfloat16, kind="Internal", addr_space="Shared").ap())
        for i in range(2)
    ]
    g3h = [
        nc.dram_tensor("g3h0", [out_f, lb_h0 * P], dt.float32,
                       kind="Internal").ap(),
        nc.dram_tensor("g3h1", [out_f, lb_h1 * P], dt.float32,
                       kind="Internal").ap(),
    ]

    from concourse import library_config

    with tile.TileContext(nc) as tc:
        with tc.tile_pool(name="dram", bufs=1, space="DRAM") as dram, \
             tc.tile_pool(name="persist", bufs=1) as persist, \
             tc.tile_pool(name="xtp", bufs=3) as xtp, \
             tc.tile_pool(name="msgp", bufs=4) as msgp, \
             tc.tile_pool(name="ohp", bufs=4) as ohp, \
             tc.tile_pool(name="htp", bufs=4) as htp, \
             tc.tile_pool(name="tmpp", bufs=2) as tmpp, \
             tc.tile_pool(name="zst", bufs=3) as zst, \
             tc.tile_pool(name="psA", bufs=3, space="PSUM") as psA, \
             tc.tile_pool(name="psB", bufs=2, space="PSUM") as psB, \
             tc.tile_pool(name="psD", bufs=2, space="PSUM") as psD:

            nc.gpsimd.load_library(library_config.mlp)
            gq = [0]          # global SWDGE queue rotation

            def next_q():
                q = gq[0] % 4
                gq[0] += 1
                return q

            # ---- persistent tiles ----
            idxwA_t = persist.tile([P, nt * Klo * 8], dt.int16, tag="gidxA")
            idxwB_t = persist.tile([P, nt * Khi * 8], dt.int16, tag="gidxB")
            idxw3_t = persist.tile([P, NG * K3 * 8], dt.int16, tag="gidx3")
            ident = persist.tile([P, P], dt.float32, tag="ident")
            HT = persist.tile([P, FCH * npad], dt.bfloat16, tag="HT")
            H3T = persist.tile([P, FCO, ncol], dt.bfloat16, tag="H3T")
            embT = persist.tile([P, ncg * P], dt.bfloat16, tag="embT")
            projT = persist.tile([P, ncol], dt.bfloat16, tag="projT")
            W1t = persist.tile([P, FCH * hid], dt.bfloat16, tag="W1")
            W2t = persist.tile([P, FCH * hid], dt.bfloat16, tag="W2")
            W3t = persist.tile([P, FCH * out_f], dt.bfloat16, tag="W3")
            Wpt = persist.tile([P, FCO * n_dim], dt.bfloat16, tag="Wp")
            b1t = persist.tile([P, FCH], dt.float32, tag="b1")
            b2t = persist.tile([P, FCH], dt.float32, tag="b2")
            b3t = persist.tile([P, FCO], dt.float32, tag="b3")
            bpt = persist.tile([P, 1], dt.float32, tag="bp")
            sc1t = persist.tile([P, nt], dt.float32, tag="sc1")
            sc23t = persist.tile([P, nt], dt.float32, tag="sc23")
            nd3t = persist.tile([P, T3], dt.float32, tag="nd3")
            cidx_t = persist.tile([P, ncg * 8], dt.int16, tag="cidx")
            e_all = persist.tile([P, ncg, n_dim], dt.float32, tag="eg")

            make_identity(nc, ident[:])
            nc.sync.dma_start(idxwA_t[:], idxwA_d[:])
            nc.sync.dma_start(idxwB_t[:], idxwB_d[:])
            nc.sync.dma_start(idxw3_t[:], idxw3_d[:])
            nc.sync.dma_start(cidx_t[:], cidx_d[:])
            nc.sync.dma_start(W1t[:], W1_d[:])
            nc.sync.dma_start(sc1t[:], sc1_d[:])
            nc.sync.dma_start(sc23t[:], sc23_d[:])
            nc.sync.dma_start(W2t[:], W2_d[:])
            nc.sync.dma_start(b1t[:], b1_d[:])
            nc.sync.dma_start(b2t[:], b2_d[:])
            nc.sync.dma_start(W3t[:], W3_d[:])
            nc.sync.dma_start(b3t[:], b3_d[:])
            nc.sync.dma_start(Wpt[:], Wp_d[:])
            nc.sync.dma_start(bpt[:], bp_d[:])
            nc.sync.dma_start(nd3t[:], nd3_d[:])

            # ---- cell-embedding gather + transpose (under AllGather-1) ----
            nc.gpsimd.dma_gather(
                e_all[:], emb_d[:], cidx_t[:], ncg * P, ncg * P, n_dim,
                queue_num=next_q())
            for g in range(ncg):
                pt = psD.tile([P, 512], dt.float32, space="PSUM", tag="pd")
                nc.tensor.transpose(pt[:, 0:P], e_all[:, g, :], ident[:])
                nc.vector.tensor_copy(embT[:, g * P:(g + 1) * P], pt[:, 0:P])

            # per-layer AllGather chunk tiles + local Z3
            zchunks = []
            for l in range(2):
                zca = dram.tile([rows_a, hid], dt.bfloat16, tag=f"zca{l}")
                zcb = dram.tile([rows_b, hid], dt.bfloat16, tag=f"zcb{l}")
                zchunks.append((zca, zcb))
            z3loc = dram.tile([npad, out_f], dt.bfloat16, tag="z3loc")
            ph0 = dram.tile([C, out_f, lb_h0 * P], dt.float32, tag="ph0")
            ph1 = dram.tile([C, out_f, lb_h1 * P], dt.float32, tag="ph1")

            def fire_ag(l, half):
                zc = zchunks[l][half]
                nc.gpsimd.collective_compute(
                    "AllGather", mybir.AluOpType.bypass,
                    replica_groups=[list(range(C))],
                    ins=[zc[:]], outs=[zfull[l][half]])

            def dense_layer(l, lhsT_of):
                """Zhat_l = scale_l * (H @ W_l); AllGather chunks (l<2) or
                local z3 (l==2)."""
                fout = hid if l < 2 else out_f
                Wt = (W1t, W2t, W3t)[l]
                sct = sc1t if l == 0 else sc23t
                for i in range(nt):
                    lhsT_tile = lhsT_of(i)
                    ps = psD.tile([P, 512], dt.float32, space="PSUM", tag="pd")
                    for f in range(FCH):
                        nc.tensor.matmul(
                            ps[:, :fout],
                            lhsT=lhsT_tile(f),
                            rhs=Wt[:, f * fout:(f + 1) * fout],
                            start=(f == 0), stop=(f == FCH - 1))
                    zs = zst.tile([P, 512], dt.bfloat16, tag="zs")
                    nc.scalar.activation(zs[:, :fout], ps[:, :fout],
                                         AF.Identity, scale=sct[:, i:i + 1])
                    if l == 2:
                        nc.sync.dma_start(z3loc[i * P:(i + 1) * P, :],
                                          zs[:, :fout])
                    elif i < nt_a:
                        nc.sync.dma_start(zchunks[l][0][i * P:(i + 1) * P, :],
                                          zs[:, :fout])
                    else:
                        j = i - nt_a
                        nc.sync.dma_start(zchunks[l][1][j * P:(j + 1) * P, :],
                                          zs[:, :fout])
                    if l < 2 and i == nt_a - 1:
                        fire_ag(l, 0)
                if l < 2:
                    fire_ag(l, 1)

            # windowed gathers: fixed 1024-row (8-tile) calls over each
            # half's tile stream, independent of bin boundaries
            WT = 8

            def make_windows(zf, idx_t, kk, tag):
                tot = nt * kk
                state = {"next": 0, "wins": {}}

                def ensure(w):
                    while state["next"] <= w:
                        i = state["next"]
                        n_t = min(WT, tot - i * WT)
                        m = msgp.tile([P, WT, FCH * P], dt.bfloat16, tag=tag)
                        nc.gpsimd.dma_gather(
                            m[:, 0:n_t, :], zf[:],
                            idx_t[:, i * WT * 8: i * WT * 8 + n_t * 8],
                            n_t * P, n_t * P, FCH * P, queue_num=next_q())
                        state["wins"][i] = m
                        state["wins"].pop(i - 4, None)
                        state["next"] += 1

                def get(j):          # j = global tile index in this half
                    ensure(j // WT)
                    return state["wins"][j // WT], j % WT

                return get

            def aggregate12(l, b_t):
                """H^T = relu(one-hot agg of gathered Zhat + b) into HT."""
                getA = make_windows(zfull[l][0], idxwA_t, Klo, "msgA")
                getB = make_windows(zfull[l][1], idxwB_t, Khi, "msgB")
                for d in range(nt):
                    oh_t = ohp.tile([P, K * P], oh_dt, tag="oh")
                    nc.sync.dma_start(oh_t[:], oh12_d[d])
                    ps = psA.tile([P, 512], dt.float32, space="PSUM", tag="pa")
                    for f in range(FCH):
                        for k in range(K):
                            if k < Klo:
                                m, s = getA(d * Klo + k)
                            else:
                                m, s = getB(d * Khi + (k - Klo))
                            nc.tensor.matmul(
                                ps[:, f * P:(f + 1) * P],
                                lhsT=m[:, s, f * P:(f + 1) * P],
                                rhs=oh_t[:, k * P:(k + 1) * P],
                                start=(k == 0), stop=(k == K - 1))
                    for f in range(FCH):
                        nc.scalar.activation(
                            HT[:, f * npad + d * P: f * npad + (d + 1) * P],
                            ps[:, f * P:(f + 1) * P],
                            AF.Relu, bias=b_t[:, f:f + 1])

            # ---- layer sequence (serial; collectives chunked A/B) ----
            def xt_lhsT(i):
                xt = xtp.tile([P, FCH * P], dt.bfloat16, tag="xt")
                nc.sync.dma_start(xt[:], xT_d[:, i * FCH * P:(i + 1) * FCH * P])
                return lambda f: xt[:, f * P:(f + 1) * P]

            def ht_lhsT(i):
                return lambda f: HT[:, f * npad + i * P: f * npad + (i + 1) * P]

            dense_layer(0, xt_lhsT)
            aggregate12(0, b1t)
            dense_layer(1, ht_lhsT)
            aggregate12(1, b2t)
            dense_layer(2, ht_lhsT)

            # ---- layer 3: src-sharded local aggregation + ReduceScatter ----
            # groups emitted lb-major: grp = lb*C + owner
            for g0 in range(0, NG, GB3):
                msg = msgp.tile([P, GB3 * K3, out_f], dt.bfloat16, tag="msg3")
                nc.gpsimd.dma_gather(
                    msg[:], z3loc[:],
                    idxw3_t[:, g0 * K3 * 8: (g0 + GB3) * K3 * 8],
                    GB3 * K3 * P, GB3 * K3 * P, out_f, queue_num=next_q())
                for j in range(GB3):
                    g = g0 + j
                    lb, ow = g // C, g % C
                    oh_t = ohp.tile([P, K3 * P], oh_dt, tag="oh3")
                    nc.sync.dma_start(oh_t[:], oh3_d[g])
                    ps = psA.tile([P, 512], dt.float32, space="PSUM", tag="pa")
                    for f in range(FCO):
                        for k in range(K3):
                            nc.tensor.matmul(
                                ps[:, f * P:(f + 1) * P],
                                lhsT=msg[:, j * K3 + k, f * P:(f + 1) * P],
                                rhs=oh_t[:, k * P:(k + 1) * P],
                                start=(k == 0), stop=(k == K3 - 1))
                    p3 = zst.tile([P, 512], dt.float32, tag="os")
                    nc.vector.tensor_copy(p3[:, :out_f], ps[:, :out_f])
                    ph, lbr = (ph0, lb) if lb < lb_h0 else (ph1, lb - lb_h0)
                    for f in range(FCO):
                        nc.sync.dma_start(
                            ph[ow, f * P:(f + 1) * P, lbr * P:(lbr + 1) * P],
                            p3[:, f * P:(f + 1) * P])
                    if g == lb_h0 * C - 1:
                        nc.gpsimd.collective_compute(
                            "ReduceScatter", mybir.AluOpType.add,
                            replica_groups=[list(range(C))],
                            ins=[ph0[:]], outs=[g3h[0]])
                    elif g == NG - 1:
                        nc.gpsimd.collective_compute(
                            "ReduceScatter", mybir.AluOpType.add,
                            replica_groups=[list(range(C))],
                            ins=[ph1[:]], outs=[g3h[1]])

            # ---- post-RS: relu(+b3) -> H3T; projection per column-half ----
            nseg = (ncol + 511) // 512
            col_base = [0, lb_h0 * P]
            for h in range(2):
                wcols = (lb_h0 if h == 0 else lb_h1) * P
                for f in range(FCO):
                    g3sb = tmpp.tile([P, 512], dt.float32, tag="g3")
                    nc.sync.dma_start(g3sb[:, :wcols],
                                      g3h[h][f * P:(f + 1) * P, :])
                    nc.scalar.activation(
                        H3T[:, f, col_base[h]:col_base[h] + wcols],
                        g3sb[:, :wcols], AF.Relu, bias=b3t[:, f:f + 1])
            for s in range(nseg):
                w = min(512, ncol - s * 512)
                pp = psD.tile([P, 512], dt.float32, space="PSUM", tag="pd")
                for f in range(FCO):
                    nc.tensor.matmul(
                        pp[:, :w],
                        lhsT=Wpt[:, f * n_dim:(f + 1) * n_dim],
                        rhs=H3T[:, f, s * 512: s * 512 + w],
                        start=(f == 0), stop=(f == FCO - 1))
                nc.scalar.activation(projT[:, s * 512:s * 512 + w], pp[:, :w],
                                     AF.Identity, bias=bpt[:, 0:1])

            # ---- outT[col, cell] = nd3[col] * projT^T @ embT ----
            ncseg = (ncg * P + 511) // 512
            for cb in range(ncol // P):
                for s in range(ncseg):
                    w = min(512, ncg * P - s * 512)
                    po = psD.tile([P, 512], dt.float32, space="PSUM", tag="pd")
                    nc.tensor.matmul(
                        po[:, :w],
                        lhsT=projT[:, cb * P:(cb + 1) * P],
                        rhs=embT[:, s * 512:s * 512 + w],
                        start=True, stop=True)
                    os_ = zst.tile([P, 512], dt.float32, tag="os")
                    nc.vector.tensor_scalar_mul(os_[:, :w], po[:, :w],
                                                nd3t[:, cb:cb + 1])
                    nc.sync.dma_start(
                        out_d[cb * P:(cb + 1) * P, s * 512:s * 512 + w],
                        os_[:, :w])

    nc.compile()
    return nc


# ----------------------------------------------------------------------------
# entry point
# ----------------------------------------------------------------------------

def _ensure_ntff_hook():
    """Register the axon NTFF-profile hook if the image's antenv lacks it.
    Only used on the TRACE path (benchmarking); grading runs trace=False."""
    import sys
    import types
    try:
        from antenv.axon_hooks import get_axon_ntff_profile_hook  # noqa: F401
        return
    except ImportError:
        pass
    try:
        from trn_agent_boot.trn_boot import _ntff_profile_via_ctypes
        hook = _ntff_profile_via_ctypes("/opt/axon/libaxon_pjrt.so")
    except Exception:
        hook = None
    mod = types.ModuleType("antenv.axon_hooks")
    mod._hook = hook
    mod.get_axon_ntff_profile_hook = lambda: mod._hook
    mod.set_axon_ntff_profile_hook = lambda h: setattr(mod, "_hook", h)
    import antenv
    antenv.axon_hooks = mod
    sys.modules["antenv.axon_hooks"] = mod


def kernel(**inputs):
    global LAST_EXEC_TIME_NS
    from concourse import bass_utils
    if TRACE:
        _ensure_ntff_hook()

    x = np.asarray(inputs["x"], np.float32)
    b1 = np.asarray(inputs["b1"], np.float32)
    b2 = np.asarray(inputs["b2"], np.float32)
    b3 = np.asarray(inputs["b3"], np.float32)
    bp = np.asarray(inputs["bp"], np.float32)
    nobias = not (np.any(b1) or np.any(b2) or np.any(b3) or np.any(bp))

    prep = preprocess(x, inputs["src"], inputs["dst"],
                      inputs["x_indices"], inputs["c_indices"], nobias)
    wp = _pack_weights(inputs["W1"], b1, inputs["W2"], b2,
                       inputs["W3"], b3, inputs["Wp"], bp,
                       inputs["emb"], inputs["c_indices"])

    hid = np.asarray(inputs["W1"]).shape[1]
    out_f = np.asarray(inputs["W3"]).shape[1]
    n_dim = np.asarray(inputs["Wp"]).shape[1]
    n_cell = np.asarray(inputs["emb"]).shape[0]
    n_c = len(np.asarray(inputs["c_indices"]))
    meta = dict(nt=prep["nt"], npad=prep["npad"],
                Klo=prep["Klo"], Khi=prep["Khi"], K3=prep["K3"],
                T3=prep["T3"], ncol=prep["ncol"], NG=prep["NG"],
                ncg=wp["ncg"], nobias=nobias, GB=GB, GB3=GB3,
                hid=hid, out_f=out_f, n_dim=n_dim, n_cell=n_cell, n_c=n_c)
    meta_key = tuple(sorted(meta.items()))
    if meta_key not in _COMPILE_CACHE:
        _COMPILE_CACHE[meta_key] = build_program(meta)
    nc = _COMPILE_CACHE[meta_key]

    in_maps = []
    for c in range(C):
        in_maps.append({
            "xT": prep["xT"][c],
            "oh12": prep["oh12"][c],
            "oh3": prep["oh3"][c],
            "idxwA": prep["idxwA"][c],
            "idxwB": prep["idxwB"][c],
            "idxw3": prep["idxw3"][c],
            "sc1": prep["sc1"][c], "sc23": prep["sc23"][c],
            "nd3": prep["nd3"][c],
            "cidx": wp["cidx"],
            "W1": wp["W1"], "W2": wp["W2"], "W3": wp["W3"], "Wp": wp["Wp"],
            "b1": wp["b1"], "b2": wp["b2"], "b3": wp["b3"], "bp": wp["bp"],
            "emb": wp["emb"],
        })

    # transient NRT_EXEC_UNIT_UNRECOVERABLE flakes recover on a fresh attempt
    last_err = None
    for _attempt in range(3):
        try:
            res = bass_utils.run_bass_kernel_spmd(
                nc, in_maps, core_ids=list(range(C)), trace=TRACE)
            break
        except Exception as e:
            last_err = e
    else:
        raise last_err
    LAST_EXEC_TIME_NS = res.exec_time_ns
    globals()["LAST_RESULTS"] = res

    outs = np.stack([r["outcT"] for r in res.results])    # [C, ncol, N_C]
    final = outs[prep["xi_owner"], prep["xi_col"], :]     # [N_SEL, N_C]
    return np.ascontiguousarray(final.T, np.float32)      # [N_C, N_SEL]


# revision 52
# speedup vs baseline: 1.4225x; 1.1934x over previous
"""Trainium2 Bass kernel for Cell2Vec GNN message passing (8 NeuronCores).

Math: 3x GraphConv (DGL norm='both') + node-select + projection + cell-embedding
scores:
    out = emb[c_indices] @ (relu-chain...)  -> [N_C, N_SEL]

Restructure used on device (per layer):
    H_next = relu( Ahat @ (H @ W) + b ),  Ahat = D_in^-1/2 A D_out^-1/2
with the degree norms folded into per-edge weights w_e = ns[src] * nd[dst].

Sharding: nodes are dst-sharded across 8 cores (6250 each, padded 6272 = 49
tiles of 128). Per layer, each core computes Z = H_own @ W (dense, PE), an
AllGather replicates Z to all cores, then each core aggregates its owned
dst-nodes: for each dst-bin (128 nodes) and edge-tile (128 edges), gather the
128 src rows of Z (indirect DMA) and accumulate on the tensor engine
    aggT[feat, dstslot] += msg[lane, feat]^T @ Onehot[lane, dstslot]
where Onehot carries w_e at (lane, dst_slot). This yields H_next^T directly
(feature-major), which is exactly the lhsT layout the next dense needs.
Layer 3 only aggregates into the x_indices-selected nodes. The final
projection + emb @ proj^T runs per-core on owned selected columns; the host
reassembles the [1024, 8192] output from per-core column blocks.

Bins are in-degree balanced per core (host preprocessing) so every bin has
the same number of edge tiles K; all 8 cores run one identical SPMD program.
"""
import heapq
import numpy as np
import ml_dtypes

P = 128
C = 8

# full-problem config (hardcoded per spec; kernel.py must be self-contained)
N_NODES = 50000
N_EDGES = 400000
IN_F = 512
HID = 512
OUT_F = 256
N_CELL = 1000
N_DIM = 128
N_SEL = 8192
N_C = 1024

BF16 = ml_dtypes.bfloat16
FP8 = ml_dtypes.float8_e4m3fn

_COMPILE_CACHE = {}
LAST_EXEC_TIME_NS = None
TRACE = False


# ----------------------------------------------------------------------------
# host preprocessing
# ----------------------------------------------------------------------------

def _balance_bins(weights, n_bins, cap):
    """Greedy balanced binning: heaviest first into least-loaded open bin.
    Returns (bin_of_item, slot_of_item)."""
    order = np.argsort(-weights, kind="stable")
    heap = [(0.0, b) for b in range(n_bins)]
    heapq.heapify(heap)
    counts = np.zeros(n_bins, np.int64)
    bin_of = np.empty(len(weights), np.int64)
    slot_of = np.empty(len(weights), np.int64)
    for i in order:
        spill = []
        while True:
            load, b = heapq.heappop(heap)
            if counts[b] < cap:
                break
            spill.append((load, b))
        bin_of[i] = b
        slot_of[i] = counts[b]
        counts[b] += 1
        heapq.heappush(heap, (load + float(weights[i]), b))
        for s in spill:
            heapq.heappush(heap, s)
    return bin_of, slot_of


def _group_edges(key, n_groups, payload_order):
    """Sort edges by group key; return per-group start/end and sorted order."""
    order = np.argsort(key, kind="stable")
    ks = key[order]
    gs = np.searchsorted(ks, np.arange(n_groups))
    ge = np.searchsorted(ks, np.arange(n_groups), side="right")
    return order, ks, gs, ge


def _build_graph_tables(isB, relrow, w_e, qslot, group_c, group_d, C_, ntiles,
                        oh_np_dt=BF16):
    """Per-(core, bin) edge layout for batched dma_gather.

    Edges of each bin are split into two gather tables (A: first row-chunk of
    every shard, B: second — matches the chunked AllGather), laid out
    A-tiles-then-B-tiles, padded to uniform global (Klo, Khi). Returns:
      Klo, Khi,
      oh   [C, ntiles, P, (Klo+Khi)*P]  bf16 one-hot (w at (lane, k*P+q)),
      idxw [C, P, ntiles*(Klo+Khi)*8]   int16 wrapped gather indices
           (per bin: Klo*8 A-columns then Khi*8 B-columns).
    """
    E = len(relrow)
    hi = np.asarray(isB).astype(np.int64)
    key = (group_c * ntiles + group_d) * 2 + hi
    order = np.argsort(key, kind="stable")
    ks = key[order]
    ngroups = C_ * ntiles * 2
    gs = np.searchsorted(ks, np.arange(ngroups))
    ge = np.searchsorted(ks, np.arange(ngroups), side="right")
    cnt = (ge - gs).reshape(C_, ntiles, 2)
    Klo = max(1, int(np.ceil(cnt[:, :, 0].max() / P)))
    Khi = max(1, int(np.ceil(cnt[:, :, 1].max() / P)))
    K = Klo + Khi

    pos = np.arange(E) - gs[ks]                  # position within (c,d,half)
    cc = ks // (2 * ntiles)
    dd = (ks // 2) % ntiles
    hh = ks % 2
    tile_ = np.where(hh == 0, pos // P, Klo + pos // P)
    lane = pos % P
    es = order

    oh = np.zeros((C_, ntiles, P, K * P), oh_np_dt)
    oh[cc, dd, lane, tile_ * P + qslot[es]] = w_e[es].astype(oh_np_dt)

    # relative int16 indices, padded slots point at row 0 (weight 0)
    ilo = np.zeros((C_, ntiles, Klo * P), np.int16)
    ihi = np.zeros((C_, ntiles, Khi * P), np.int16)
    mlo, mhi = hh == 0, hh == 1
    ilo[cc[mlo], dd[mlo], pos[mlo]] = relrow[es[mlo]].astype(np.int16)
    ihi[cc[mhi], dd[mhi], pos[mhi]] = relrow[es[mhi]].astype(np.int16)

    def wrap(v):   # [..., L] -> [..., 16, L//16] with unwrapped[j] = w[j%16, j//16]
        shp = v.shape[:-1]
        L = v.shape[-1]
        return v.reshape(*shp, L // 16, 16).swapaxes(-1, -2)

    wlo = wrap(ilo)                               # [C, ntiles, 16, Klo*8]
    whi = wrap(ihi)
    percol = np.concatenate([wlo, whi], axis=-1)  # [C, ntiles, 16, K*8]
    percol = percol.transpose(0, 2, 1, 3).reshape(C_, 16, ntiles * K * 8)
    idxw = np.tile(percol, (1, P // 16, 1))       # replicate to 128 partitions
    return Klo, Khi, np.ascontiguousarray(oh), np.ascontiguousarray(idxw)


def preprocess(x, src, dst, x_indices, c_indices, nobias):
    src = np.asarray(src).astype(np.int64)
    dst = np.asarray(dst).astype(np.int64)
    x_indices = np.asarray(x_indices).astype(np.int64)
    c_indices = np.asarray(c_indices).astype(np.int64)
    x = np.asarray(x)
    n = x.shape[0]
    nshard = n // C
    nt = (nshard + P - 1) // P
    npad = nt * P
    nt_a = (nt + 1) // 2           # AllGather chunk A = first nt_a tiles
    rows_a, rows_b = nt_a * P, (nt - nt_a) * P

    deg_out = np.bincount(src, minlength=n).astype(np.float64)
    deg_in = np.bincount(dst, minlength=n).astype(np.float64)
    ns = np.where(deg_out > 0, 1.0 / np.sqrt(np.maximum(deg_out, 1.0)), 0.0)
    nd = np.where(deg_in > 0, 1.0 / np.sqrt(np.maximum(deg_in, 1.0)), 0.0)
    w_e = (ns[src] * nd[dst]).astype(np.float32)

    owner_n = np.arange(n) // nshard
    localrow = np.empty(n, np.int64)
    for c in range(C):
        nodes = np.arange(c * nshard, (c + 1) * nshard)
        b, s = _balance_bins(deg_in[nodes], nt, P)
        localrow[nodes] = b * P + s
    # chunked-AllGather relative row: table A holds rows [0, rows_a) of every
    # shard (concatenated by owner), table B the rest.
    isB_n = localrow >= rows_a
    relrow_n = np.where(isB_n, owner_n * rows_b + (localrow - rows_a),
                        owner_n * rows_a + localrow)

    # L1/L2 edge layout. With zero biases relu(nd*G) = nd*relu(G), so the
    # degree norms fold into EXACT fp32 per-node scales at the Z-cast and the
    # one-hot becomes binary 1.0 stored in fp8 (half the table bytes).
    w12 = np.ones_like(w_e) if nobias else w_e
    Klo, Khi, oh12, idxw12 = _build_graph_tables(
        isB_n[src], relrow_n[src], w12, (localrow[dst] % P).astype(np.int64),
        dst // nshard, localrow[dst] // P, C, nt,
        FP8 if nobias else BF16)

    # per-node Z-cast scales, bin layout [C, 128, nt]
    sc1_n = ns if nobias else np.ones(n)
    sc23_n = (ns * nd) if nobias else np.ones(n)
    sc1 = np.zeros((C, P, nt), np.float32)
    sc23 = np.zeros((C, P, nt), np.float32)
    for c in range(C):
        nodes = np.arange(c * nshard, (c + 1) * nshard)
        lr = localrow[nodes]
        sc1[c, lr % P, lr // P] = sc1_n[nodes]
        sc23[c, lr % P, lr // P] = sc23_n[nodes]

    # L3: selected nodes only
    sel_nodes = np.unique(x_indices)
    sel_mask = np.zeros(n, bool)
    sel_mask[sel_nodes] = True
    e3 = np.nonzero(sel_mask[dst])[0]
    deg3 = np.bincount(dst[e3], minlength=n).astype(np.float64)
    ncol_max = max(int((sel_nodes // nshard == c).sum()) for c in range(C))
    T3 = max(1, (ncol_max + P - 1) // P)
    ncol = T3 * P
    colpos = np.full(n, 0, np.int64)
    for c in range(C):
        nodes = sel_nodes[sel_nodes // nshard == c]
        b, s = _balance_bins(deg3[nodes], T3, P)
        colpos[nodes] = b * P + s
    # with nobias scales, Zhat3 already carries ns[src] (and the src-side nd),
    # so the L3 edge weight reduces to nd[dst]
    w3 = nd[dst[e3]].astype(np.float32) if nobias else w_e[e3]
    K3lo, K3hi, oh3, idxw3 = _build_graph_tables(
        isB_n[src[e3]], relrow_n[src[e3]], w3,
        (colpos[dst[e3]] % P).astype(np.int64),
        dst[e3] // nshard, colpos[dst[e3]] // P, C, T3)

    xi_owner = (x_indices // nshard).astype(np.int32)
    xi_col = colpos[x_indices].astype(np.int32)

    # per-core permuted x^T in [128, 4, npad] chunk layout
    F = x.shape[1]
    FC = F // P
    xT = np.zeros((C, P, FC, npad), BF16)
    for c in range(C):
        nodes = np.arange(c * nshard, (c + 1) * nshard)
        xv = x[nodes].astype(BF16)            # [nshard, F]
        for fc in range(FC):
            xT[c, :, fc, localrow[nodes]] = xv[:, fc * P:(fc + 1) * P]
    xT = xT.reshape(C, P, FC * npad)

    return dict(
        n=n, nshard=nshard, nt=nt, npad=npad, T3=T3, ncol=ncol,
        Klo=Klo, Khi=Khi, K3lo=K3lo, K3hi=K3hi,
        idxw12=idxw12, idxw3=idxw3, oh12=oh12, oh3=oh3, xT=xT,
        sc1=sc1, sc23=sc23,
        xi_owner=xi_owner, xi_col=xi_col,
    )


def _pack_weights(W1, b1, W2, b2, W3, b3, Wp, bp, emb, c_indices):
    """Device layouts: W [fin, fout] -> [128, nchunk*fout]; b -> [128, nchunk]."""
    def wdev(W):
        fin, fout = W.shape
        nc_ = fin // P
        return np.ascontiguousarray(
            W.astype(BF16).reshape(nc_, P, fout).transpose(1, 0, 2).reshape(P, nc_ * fout))

    def bdev(b):
        nc_ = len(b) // P
        return np.ascontiguousarray(
            np.asarray(b, np.float32).reshape(nc_, P).T)

    c_idx = np.asarray(c_indices, np.int64)
    ncg = (len(c_idx) + P - 1) // P
    tmp = np.zeros(ncg * P, np.int16)
    tmp[:len(c_idx)] = c_idx
    # wrapped int16 for dma_gather: idx j at [j % 16, j // 16], replicated x8
    cidx_dev = np.ascontiguousarray(
        np.tile(tmp.reshape(ncg * 8, 16).T, (P // 16, 1)))
    return dict(
        W1=wdev(W1), W2=wdev(W2), W3=wdev(W3), Wp=wdev(Wp),
        b1=bdev(b1), b2=bdev(b2), b3=bdev(b3), bp=bdev(bp),
        emb=np.asarray(emb, np.float32), cidx=cidx_dev, ncg=ncg,
    )


# ----------------------------------------------------------------------------
# bass program
# ----------------------------------------------------------------------------

def build_program(meta):
    import concourse.bacc as bacc
    import concourse.bass as bass
    import concourse.mybir as mybir
    import concourse.tile as tile
    from concourse.masks import make_identity

    nt, npad = meta["nt"], meta["npad"]
    T3, ncol = meta["T3"], meta["ncol"]
    Klo, Khi = meta["Klo"], meta["Khi"]
    K3lo, K3hi = meta["K3lo"], meta["K3hi"]
    K = Klo + Khi
    K3 = K3lo + K3hi
    ncg = meta["ncg"]
    hid, out_f = meta["hid"], meta["out_f"]
    n_cell, n_dim, n_c = meta["n_cell"], meta["n_dim"], meta["n_c"]
    nt_a = (nt + 1) // 2
    nt_b = nt - nt_a
    rows_a, rows_b = nt_a * P, nt_b * P
    FCH = hid // P            # chunks of hidden width
    FCO = out_f // P          # chunks of layer-3 output width
    dt = mybir.dt
    AF = mybir.ActivationFunctionType
    oh12_dt = dt.float8e4 if meta["nobias"] else dt.bfloat16

    nc = bacc.Bacc("TRN2", target_bir_lowering=False, debug=False, num_devices=C,
                   num_swdge_queues=4)

    def din(name, shape, dtype):
        return nc.dram_tensor(name, list(shape), dtype, kind="ExternalInput").ap()

    xT_d = din("xT", (P, FCH * npad), dt.bfloat16)
    oh12_d = din("oh12", (nt, P, K * P), oh12_dt)
    oh3_d = din("oh3", (T3, P, K3 * P), dt.bfloat16)
    sc1_d = din("sc1", (P, nt), dt.float32)
    sc23_d = din("sc23", (P, nt), dt.float32)
    idxw12_d = din("idxw12", (P, nt * K * 8), dt.int16)
    idxw3_d = din("idxw3", (P, T3 * K3 * 8), dt.int16)
    cidx_d = din("cidx", (P, ncg * 8), dt.int16)
    W1_d = din("W1", (P, FCH * hid), dt.bfloat16)
    W2_d = din("W2", (P, FCH * hid), dt.bfloat16)
    W3_d = din("W3", (P, FCH * out_f), dt.bfloat16)
    Wp_d = din("Wp", (P, FCO * n_dim), dt.bfloat16)
    b1_d = din("b1", (P, FCH), dt.float32)
    b2_d = din("b2", (P, FCH), dt.float32)
    b3_d = din("b3", (P, FCO), dt.float32)
    bp_d = din("bp", (P, 1), dt.float32)
    emb_d = din("emb", (n_cell, n_dim), dt.float32)

    out_d = nc.dram_tensor("outc", [n_c, ncol], dt.float32, kind="ExternalOutput").ap()
    if meta.get("debug"):
        dbg_h3 = nc.dram_tensor("dbg_h3", [P, (out_f // P) * ncol], dt.bfloat16,
                                kind="ExternalOutput").ap()
        dbg_proj = nc.dram_tensor("dbg_proj", [P, ncol], dt.bfloat16,
                                  kind="ExternalOutput").ap()
        dbg_emb = nc.dram_tensor("dbg_emb", [P, ncg * P], dt.bfloat16,
                                 kind="ExternalOutput").ap()

    zfull = [
        (nc.dram_tensor(f"zfullA{i}", [C * rows_a, hid if i < 2 else out_f],
                        dt.bfloat16, kind="Internal", addr_space="Shared").ap(),
         nc.dram_tensor(f"zfullB{i}", [C * rows_b, hid if i < 2 else out_f],
                        dt.bfloat16, kind="Internal", addr_space="Shared").ap())
        for i in range(3)
    ]

    from concourse import library_config

    with tile.TileContext(nc) as tc:
        with tc.tile_pool(name="dram", bufs=1, space="DRAM") as dram, \
             tc.tile_pool(name="persist", bufs=1) as persist, \
             tc.tile_pool(name="wpool", bufs=1) as wpool, \
             tc.tile_pool(name="sbuf", bufs=3) as sbuf, \
             tc.tile_pool(name="msgp", bufs=3) as msgp, \
             tc.tile_pool(name="ohp", bufs=3) as ohp, \
             tc.tile_pool(name="zst", bufs=3) as zst, \
             tc.tile_pool(name="psum_d", bufs=2, space="PSUM") as psum_d, \
             tc.tile_pool(name="psum_a", bufs=4, space="PSUM") as psum_a:

            nc.gpsimd.load_library(library_config.mlp)
            gq = [0]          # global SWDGE queue rotation (lane i <-> queue i%4)

            def next_q():
                q = gq[0] % 4
                gq[0] += 1
                return q

            # persistent tiles
            HT_a = persist.tile([P, FCH * npad], dt.bfloat16, tag="HT_a")
            HT_b = persist.tile([P, FCH * npad], dt.bfloat16, tag="HT_b")
            H3T = persist.tile([P, FCO * ncol], dt.bfloat16, tag="H3T")
            idxw12_t = persist.tile([P, nt * K * 8], dt.int16, tag="gidx")
            idxw3_t = persist.tile([P, T3 * K3 * 8], dt.int16, tag="gidx3")
            ident = persist.tile([P, P], dt.float32, tag="ident")
            make_identity(nc, ident[:])

            sc1t = persist.tile([P, nt], dt.float32, tag="sc1")
            sc23t = persist.tile([P, nt], dt.float32, tag="sc23")
            nc.sync.dma_start(sc1t[:], sc1_d[:])
            nc.sync.dma_start(sc23t[:], sc23_d[:])
            nc.sync.dma_start(HT_a[:], xT_d[:])
            nc.sync.dma_start(idxw12_t[:], idxw12_d[:])
            nc.sync.dma_start(idxw3_t[:], idxw3_d[:])

            def dense(HT, W_ap, fout, zf_idx, sct):
                """Z = H_own @ W -> DRAM (bf16, node-major), AllGather in two
                row-chunks so aggregation can start after chunk A lands."""
                Wt = wpool.tile([P, FCH * fout], dt.bfloat16, tag="W")
                nc.sync.dma_start(Wt[:], W_ap[:])
                zca = dram.tile([rows_a, fout], dt.bfloat16, tag=f"zca{zf_idx}")
                zcb = dram.tile([rows_b, fout], dt.bfloat16, tag=f"zcb{zf_idx}")
                for i in range(nt):
                    ps = psum_d.tile([P, fout], dt.float32, space="PSUM", tag="pd")
                    for f in range(FCH):
                        nc.tensor.matmul(
                            ps[:],
                            lhsT=HT[:, f * npad + i * P: f * npad + (i + 1) * P],
                            rhs=Wt[:, f * fout:(f + 1) * fout],
                            start=(f == 0), stop=(f == FCH - 1))
                    zs = zst.tile([P, fout], dt.bfloat16, tag="zs")
                    nc.scalar.activation(zs[:], ps[:], AF.Identity,
                                         scale=sct[:, i:i + 1])
                    if i < nt_a:
                        nc.sync.dma_start(zca[i * P:(i + 1) * P, :], zs[:])
                    else:
                        j = i - nt_a
                        nc.sync.dma_start(zcb[j * P:(j + 1) * P, :], zs[:])
                    if i == nt_a - 1:
                        nc.gpsimd.collective_compute(
                            "AllGather", mybir.AluOpType.bypass,
                            replica_groups=[list(range(C))],
                            ins=[zca[:]], outs=[zfull[zf_idx][0]])
                nc.gpsimd.collective_compute(
                    "AllGather", mybir.AluOpType.bypass,
                    replica_groups=[list(range(C))],
                    ins=[zcb[:]], outs=[zfull[zf_idx][1]])

            def aggregate(zf_idx, oh_ap, idx_t, b_ap, HTout, ntiles, klo, khi,
                          fch, oh_tile_dt):
                """H_out^T[:, bin] = relu( sum_k msg_k^T @ oh_k + b ).
                Per bin: two batched dma_gathers (lo/hi half of zfull)."""
                kt = klo + khi
                elem = fch * P
                bt = wpool.tile([P, fch], dt.float32, tag="b")
                nc.sync.dma_start(bt[:], b_ap[:])
                zfa, zfb = zfull[zf_idx]
                for d in range(ntiles):
                    oh_t = ohp.tile([P, kt * P], oh_tile_dt,
                                    tag="oh12" if zf_idx < 2 else "oh3")
                    nc.sync.dma_start(oh_t[:], oh_ap[d])
                    ps = psum_a.tile([P, fch * P], dt.float32, space="PSUM", tag="pa")
                    msg = msgp.tile([P, kt, elem], dt.bfloat16, tag="msg")
                    icol = d * kt * 8
                    nc.gpsimd.dma_gather(
                        msg[:, 0:klo, :], zfa[:],
                        idx_t[:, icol: icol + klo * 8],
                        klo * P, klo * P, elem, queue_num=next_q())
                    nc.gpsimd.dma_gather(
                        msg[:, klo:kt, :], zfb[:],
                        idx_t[:, icol + klo * 8: icol + kt * 8],
                        khi * P, khi * P, elem, queue_num=next_q())
                    for f in range(fch):
                        for k in range(kt):
                            nc.tensor.matmul(
                                ps[:, f * P:(f + 1) * P],
                                lhsT=msg[:, k, f * P:(f + 1) * P],
                                rhs=oh_t[:, k * P:(k + 1) * P],
                                start=(k == 0), stop=(k == kt - 1))
                    for f in range(fch):
                        nc.scalar.activation(
                            HTout[:, f * (ntiles * P) + d * P:
                                  f * (ntiles * P) + (d + 1) * P],
                            ps[:, f * P:(f + 1) * P],
                            AF.Relu, bias=bt[:, f:f + 1])

            # ---- layers ----
            dense(HT_a, W1_d, hid, 0, sc1t)
            aggregate(0, oh12_d, idxw12_t, b1_d, HT_b, nt, Klo, Khi, FCH,
                      oh12_dt)
            dense(HT_b, W2_d, hid, 1, sc23t)
            aggregate(1, oh12_d, idxw12_t, b2_d, HT_a, nt, Klo, Khi, FCH,
                      oh12_dt)
            dense(HT_a, W3_d, out_f, 2, sc23t)
            aggregate(2, oh3_d, idxw3_t, b3_d, H3T, T3, K3lo, K3hi, FCO,
                      dt.bfloat16)

            # ---- projection: projT = Wp^T @ enc^T + bp  [n_dim, ncol] ----
            Wpt = wpool.tile([P, FCO * n_dim], dt.bfloat16, tag="W")
            bpt = wpool.tile([P, 1], dt.float32, tag="b")
            nc.sync.dma_start(Wpt[:], Wp_d[:])
            nc.sync.dma_start(bpt[:], bp_d[:])
            projT = persist.tile([P, ncol], dt.bfloat16, tag="projT")
            nseg = (ncol + 511) // 512
            for s in range(nseg):
                w = min(512, ncol - s * 512)
                pp = psum_d.tile([P, 512], dt.float32, space="PSUM", tag="pd")
                for f in range(FCO):
                    nc.tensor.matmul(
                        pp[:, :w],
                        lhsT=Wpt[:, f * n_dim:(f + 1) * n_dim],
                        rhs=H3T[:, f * ncol + s * 512: f * ncol + s * 512 + w],
                        start=(f == 0), stop=(f == FCO - 1))
                nc.scalar.activation(projT[:, s * 512:s * 512 + w], pp[:, :w],
                                     AF.Identity, bias=bpt[:, 0:1])

            # ---- EmbSel^T: gather emb[c_indices] and transpose ----
            cidx_t = sbuf.tile([P, ncg * 8], dt.int16, tag="cidx")
            nc.sync.dma_start(cidx_t[:], cidx_d[:])
            embT = persist.tile([P, ncg * P], dt.bfloat16, tag="embT")
            e_all = sbuf.tile([P, ncg, n_dim], dt.float32, tag="eg")
            nc.gpsimd.dma_gather(
                e_all[:], emb_d[:], cidx_t[:], ncg * P, ncg * P, n_dim,
                queue_num=next_q())
            for g in range(ncg):
                pt = psum_d.tile([P, P], dt.float32, space="PSUM", tag="pd")
                nc.tensor.transpose(pt[:], e_all[:, g, :], ident[:])
                nc.vector.tensor_copy(embT[:, g * P:(g + 1) * P], pt[:])

            if meta.get("debug"):
                nc.sync.dma_start(dbg_h3[:], H3T[:])
                nc.sync.dma_start(dbg_proj[:], projT[:])
                nc.sync.dma_start(dbg_emb[:], embT[:])

            # ---- out_c = EmbSel @ projT  [N_C, ncol] ----
            for g in range(ncg):
                for s in range(nseg):
                    w = min(512, ncol - s * 512)
                    po = psum_d.tile([P, 512], dt.float32, space="PSUM", tag="pd")
                    nc.tensor.matmul(
                        po[:, :w],
                        lhsT=embT[:, g * P:(g + 1) * P],
                        rhs=projT[:, s * 512:s * 512 + w],
                        start=True, stop=True)
                    os_ = zst.tile([P, 512], dt.float32, tag="os")
                    nc.vector.tensor_copy(os_[:, :w], po[:, :w])
                    nc.sync.dma_start(
                        out_d[g * P:(g + 1) * P, s * 512:s * 512 + w],
                        os_[:, :w])

    nc.compile()
    return nc


# ----------------------------------------------------------------------------
# entry point
# ----------------------------------------------------------------------------

def _ensure_ntff_hook():
    """Register the axon NTFF-profile hook if the image's antenv lacks it.
    Only used on the TRACE path (benchmarking); grading runs trace=False."""
    import sys
    import types
    try:
        from antenv.axon_hooks import get_axon_ntff_profile_hook  # noqa: F401
        return
    except ImportError:
        pass
    try:
        from trn_agent_boot.trn_boot import _ntff_profile_via_ctypes
        hook = _ntff_profile_via_ctypes("/opt/axon/libaxon_pjrt.so")
    except Exception:
        hook = None
    mod = types.ModuleType("antenv.axon_hooks")
    mod._hook = hook
    mod.get_axon_ntff_profile_hook = lambda: mod._hook
    mod.set_axon_ntff_profile_hook = lambda h: setattr(mod, "_hook", h)
    import antenv
    antenv.axon_hooks = mod
    sys.modules["antenv.axon_hooks"] = mod


def kernel(**inputs):
    global LAST_EXEC_TIME_NS
    from concourse import bass_utils
    if TRACE:
        _ensure_ntff_hook()

    x = np.asarray(inputs["x"], np.float32)
    nobias = not (np.any(np.asarray(inputs["b1"]))
                  or np.any(np.asarray(inputs["b2"]))
                  or np.any(np.asarray(inputs["b3"])))
    prep = preprocess(x, inputs["src"], inputs["dst"],
                      inputs["x_indices"], inputs["c_indices"], nobias)
    wp = _pack_weights(inputs["W1"], inputs["b1"], inputs["W2"], inputs["b2"],
                       inputs["W3"], inputs["b3"], inputs["Wp"], inputs["bp"],
                       inputs["emb"], inputs["c_indices"])

    hid = np.asarray(inputs["W1"]).shape[1]
    out_f = np.asarray(inputs["W3"]).shape[1]
    n_dim = np.asarray(inputs["Wp"]).shape[1]
    n_cell = np.asarray(inputs["emb"]).shape[0]
    n_c = len(np.asarray(inputs["c_indices"]))
    meta = dict(nt=prep["nt"], npad=prep["npad"],
                Klo=prep["Klo"], Khi=prep["Khi"],
                K3lo=prep["K3lo"], K3hi=prep["K3hi"],
                T3=prep["T3"], ncol=prep["ncol"], ncg=wp["ncg"],
                nobias=nobias,
                hid=hid, out_f=out_f, n_dim=n_dim, n_cell=n_cell, n_c=n_c)
    meta_key = tuple(sorted(meta.items()))
    if meta_key not in _COMPILE_CACHE:
        _COMPILE_CACHE[meta_key] = build_program(meta)
    nc = _COMPILE_CACHE[meta_key]

    in_maps = []
    for c in range(C):
        in_maps.append({
            "xT": prep["xT"][c],
            "oh12": prep["oh12"][c],
            "oh3": prep["oh3"][c],
            "idxw12": prep["idxw12"][c],
            "idxw3": prep["idxw3"][c],
            "sc1": prep["sc1"][c], "sc23": prep["sc23"][c],
            "cidx": wp["cidx"],
            "W1": wp["W1"], "W2": wp["W2"], "W3": wp["W3"], "Wp": wp["Wp"],
            "b1": wp["b1"], "b2": wp["b2"], "b3": wp["b3"], "bp": wp["bp"],
            "emb": wp["emb"],
        })

    # transient NRT_EXEC_UNIT_UNRECOVERABLE flakes recover on a fresh attempt
    last_err = None
    for _attempt in range(3):
        try:
            res = bass_utils.run_bass_kernel_spmd(
                nc, in_maps, core_ids=list(range(C)), trace=TRACE)
            break
        except Exception as e:
            last_err = e
    else:
        raise last_err
    LAST_EXEC_TIME_NS = res.exec_time_ns
    globals()["LAST_RESULTS"] = res

    outs = np.stack([r["outc"] for r in res.results])     # [C, N_C, ncol]
    final = outs[prep["xi_owner"], :, prep["xi_col"]]     # [N_SEL, N_C]
    return np.ascontiguousarray(final.T, np.float32)      # [N_C, N_SEL]

